# revision 2
# baseline (speedup 1.0000x reference)
"""Trainium2 Bass kernel for nn_Hierarch_RNN (hierarchical 2-layer GRU), v2.

Changes vs v1 baseline:
  - bf16 for all matmul operands + elementwise state (PSUM stays f32):
    2x DVE throughput on SBUF tensor-tensor ops, halved SBUF/DMA traffic,
    same PE rate as f32r.
  - Layer-1 input is periodic with period 4 (segments 0..3 repeat 15x),
    so the x-side gate pre-activations gi = Wih1 @ relu(emb) are computed
    once per unique segment and cached in SBUF; the 56 repeated steps
    initialize the r/z PSUM with an identity matmul from the cache and the
    n-gate folds the cached term into the existing DVE add. Saves 30 of 78
    matmuls per L1 step and all embed work.
  - Embed relu moved from ScalarE to a fused DVE tensor_scalar
    (add-bias, max-0) to balance engine load (ScalarE was near-critical).
  - L1 tanh batched over the full row dim (one act per 128-block instead
    of three) to amortize the ~350-cycle ACT instruction overhead.
"""
import numpy as np

import concourse.mybir as mybir
import concourse.tile as tile
from concourse import bacc
from concourse.bass_utils import run_bass_kernel_spmd

F32 = mybir.dt.float32
BF = mybir.dt.bfloat16
AF = mybir.ActivationFunctionType
ALU = mybir.AluOpType

B, SEQ, PRED, ENC = 32, 720, 96, 321
NCORE, BPC = 8, 4
R = BPC * ENC                 # 1284 rows per core
CH, NCH = 428, 3              # row chunks (428 f32 <= 512/bank)
# layer params: d, seg_len, n 128-blocks of d (DK), decoder steps S
D0, SG0, DK0, S0, T0 = 512, 48, 4, 2, 15
D1, SG1, DK1, S1, T1 = 256, 24, 2, 4, 60

_CACHE = {}

# Packed-input layouts: all bf16 tensors in one flat blob, all f32 in
# another — per-argument dispatch overhead through the exec path is large,
# so the kernel takes 2 inputs + 1 output instead of ~31/2.
# posx ships unreplicated ([.., 321] instead of [.., R=4*321]); the 4x
# batch replication happens via 4 on-device DMA reads.
PACK_BF = [
    ("xseg0", (T0, SG0, R)),
    ("xseg1", (SG1, 4 * R)),
    ("wihT0", (DK0, 128, 3 * D0)), ("wihT1", (DK1, 128, 3 * D1)),
    ("whhT0", (DK0, 128, 3 * D0)), ("whhT1", (DK1, 128, 3 * D1)),
    ("wembT0", (SG0, D0)), ("wembT1", (SG1, D1)),
    ("wpredT0", (DK0, 128, SG0)), ("wpredT1", (DK1, 128, SG1)),
    ("posx0", (S0, DK0, 128, ENC)), ("posx1", (S1, DK1, 128, ENC)),
    ("ident", (128, 128)),
]
PACK_F32 = [
    ("brz0", (128, 2 * DK0)), ("brz1", (128, 2 * DK1)),
    ("bihn0", (128, DK0)), ("bihn1", (128, DK1)),
    ("bhhn0", (128, DK0)), ("bhhn1", (128, DK1)),
    ("bemb0", (128, DK0)), ("bemb1", (128, DK1)),
    ("bpred0", (128, 1)), ("bpred1", (128, 1)),
]


def _offsets(spec):
    out, off = {}, 0
    for name, shape in spec:
        n = int(np.prod(shape))
        out[name] = (off, shape)
        off += n
    return out, off


OFF_BF, N_BF = _offsets(PACK_BF)
OFF_F32, N_F32 = _offsets(PACK_F32)
NY0, NY1 = S0 * SG0 * R, S1 * SG1 * R


def _build_nc(l0_steps=T0, l1_steps=T1):
    nc = bacc.Bacc("TRN2", target_bir_lowering=False, debug=False,
                   num_devices=NCORE)

    # ---------------- DRAM tensors (packed) ----------------
    bb_d = nc.dram_tensor("bb", [N_BF], BF, kind="ExternalInput")
    bf_d = nc.dram_tensor("bf", [N_F32], F32, kind="ExternalInput")
    yy_d = nc.dram_tensor("yy", [NY0 + NY1], F32, kind="ExternalOutput")

    def vbf(name):
        off, shape = OFF_BF[name]
        n = int(np.prod(shape))
        ap = bb_d[off:off + n]
        if len(shape) == 2:
            return ap.rearrange("(a b) -> a b", a=shape[0])
        if len(shape) == 3:
            return ap.rearrange("(a b c) -> a b c", a=shape[0], b=shape[1])
        return ap.rearrange("(a b c d) -> a b c d",
                            a=shape[0], b=shape[1], c=shape[2])

    def vf32(name):
        off, shape = OFF_F32[name]
        n = int(np.prod(shape))
        return bf_d[off:off + n].rearrange("(a b) -> a b", a=shape[0])

    xseg0_d = vbf("xseg0")
    xseg1_d = vbf("xseg1")
    wih_d = [vbf("wihT0"), vbf("wihT1")]
    whh_d = [vbf("whhT0"), vbf("whhT1")]
    wemb_d = [vbf("wembT0"), vbf("wembT1")]
    wpred_d = [vbf("wpredT0"), vbf("wpredT1")]
    brz_d = [vf32("brz0"), vf32("brz1")]
    bihn_d = [vf32("bihn0"), vf32("bihn1")]
    bhhn_d = [vf32("bhhn0"), vf32("bhhn1")]
    bemb_d = [vf32("bemb0"), vf32("bemb1")]
    bpred_d = [vf32("bpred0"), vf32("bpred1")]
    posx_d = [vbf("posx0"), vbf("posx1")]
    ident_d = vbf("ident")
    y_d = [yy_d[0:NY0].rearrange("(a b c) -> a b c", a=S0, b=SG0),
           yy_d[NY0:NY0 + NY1].rearrange("(a b c) -> a b c", a=S1, b=SG1)]

    with tile.TileContext(nc) as tc:
        with tc.tile_pool(name="const", bufs=1) as cp, \
             tc.tile_pool(name="x0p", bufs=2) as x0p, \
             tc.tile_pool(name="xep", bufs=6) as xep, \
             tc.tile_pool(name="h0p", bufs=8) as h0p, \
             tc.tile_pool(name="h1p", bufs=4) as h1p, \
             tc.tile_pool(name="posp", bufs=5) as posp, \
             tc.tile_pool(name="hyp", bufs=6) as hyp, \
             tc.tile_pool(name="rp", bufs=4) as rp, \
             tc.tile_pool(name="zp", bufs=4) as zp, \
             tc.tile_pool(name="np_", bufs=4) as np_p, \
             tc.tile_pool(name="scp", bufs=4) as scp, \
             tc.tile_pool(name="tp", bufs=4) as tp, \
             tc.tile_pool(name="up", bufs=4) as up, \
             tc.tile_pool(name="vp", bufs=4) as vp, \
             tc.tile_pool(name="yp", bufs=2) as yp, \
             tc.tile_pool(name="psg", bufs=6, space="PSUM") as psg, \
             tc.tile_pool(name="psy", bufs=2, space="PSUM") as psy:

            # ---------------- load constants ----------------
            def load_w(dram, k_tiles, cols, nm):
                t = cp.tile([128, k_tiles * cols], BF, tag=f"c_{nm}",
                            name=f"c_{nm}")
                for k in range(k_tiles):
                    nc.sync.dma_start(t[:, k * cols:(k + 1) * cols], dram[k])
                return t

            wih_sb = [load_w(wih_d[0], DK0, 3 * D0, "wih0"),
                      load_w(wih_d[1], DK1, 3 * D1, "wih1")]
            whh_sb = [load_w(whh_d[0], DK0, 3 * D0, "whh0"),
                      load_w(whh_d[1], DK1, 3 * D1, "whh1")]
            wpred_sb = [load_w(wpred_d[0], DK0, SG0, "wpred0"),
                        load_w(wpred_d[1], DK1, SG1, "wpred1")]
            wemb_sb = []
            for li, (sg, d) in enumerate(((SG0, D0), (SG1, D1))):
                t = cp.tile([sg, d], BF, tag=f"c_wemb{li}", name=f"c_wemb{li}")
                nc.sync.dma_start(t[:], wemb_d[li][:])
                wemb_sb.append(t)
            def load_b(dram, cols, nm):
                t = cp.tile([128, cols], F32, tag=f"c_{nm}", name=f"c_{nm}")
                nc.sync.dma_start(t[:], dram[:])
                return t
            brz_sb = [load_b(brz_d[0], 2 * DK0, "brz0"), load_b(brz_d[1], 2 * DK1, "brz1")]
            bihn_sb = [load_b(bihn_d[0], DK0, "bihn0"), load_b(bihn_d[1], DK1, "bihn1")]
            bhhn_sb = [load_b(bhhn_d[0], DK0, "bhhn0"), load_b(bhhn_d[1], DK1, "bhhn1")]
            bemb_sb = [load_b(bemb_d[0], DK0, "bemb0"), load_b(bemb_d[1], DK1, "bemb1")]
            bpred_sb = [load_b(bpred_d[0], 1, "bpred0"), load_b(bpred_d[1], 1, "bpred1")]
            xs1 = cp.tile([SG1, 4 * R], BF, tag="c_xs1", name="c_xs1")
            nc.sync.dma_start(xs1[:], xseg1_d[:])
            ident = cp.tile([128, 128], BF, tag="c_ident", name="c_ident")
            nc.sync.dma_start(ident[:], ident_d[:])
            # L1 x-side gate cache: gi1[j][m] = (Wih1 @ relu(emb(xs1_j)))[m]
            # for the 4 unique input segments, m over the 6 output 128-blocks.
            n_var = min(4, l1_steps)
            gi1 = [[cp.tile([128, R], BF, tag=f"c_gi1_{j}_{m}",
                            name=f"c_gi1_{j}_{m}")
                    for m in range(3 * DK1)] for j in range(n_var)]

            LP = [dict(D=D0, DK=DK0, SG=SG0, wih=wih_sb[0], whh=whh_sb[0],
                       wemb=wemb_sb[0], wpred=wpred_sb[0], brz=brz_sb[0],
                       bihn=bihn_sb[0], bhhn=bhhn_sb[0], bemb=bemb_sb[0],
                       bpred=bpred_sb[0]),
                  dict(D=D1, DK=DK1, SG=SG1, wih=wih_sb[1], whh=whh_sb[1],
                       wemb=wemb_sb[1], wpred=wpred_sb[1], brz=brz_sb[1],
                       bihn=bihn_sb[1], bhhn=bhhn_sb[1], bemb=bemb_sb[1],
                       bpred=bpred_sb[1])]

            def wcol(P, wt, k, m):
                """AP of [128,128] weight block: k-tile k, m-tile m of 3d."""
                c0 = k * 3 * P["D"] + m * 128
                return wt[:, c0:c0 + 128]

            def make_xe_embed(li, xsrc_fn):
                """Returns make_xe(c): emits per-chunk embed, returns DK APs."""
                P = LP[li]
                def make_xe(c):
                    aps = []
                    for k in range(P["DK"]):
                        ps = psg.tile([128, CH], F32, tag="ps", name="ps_e")
                        nc.tensor.matmul(ps[:], P["wemb"][:, k * 128:(k + 1) * 128],
                                         xsrc_fn(c), start=True, stop=True)
                        xe = xep.tile([128, CH], BF, tag="xe", name=f"xe{li}_{k}")
                        # relu(x + bemb) fused on DVE (offloads ScalarE)
                        nc.vector.tensor_scalar(xe[:], ps[:],
                                                P["bemb"][:, k:k + 1], 0.0,
                                                op0=ALU.add, op1=ALU.max)
                        aps.append(xe[:])
                    return aps
                return make_xe

            def emit_gru(li, make_xe, h_in, hout, first, gi_cache=None):
                """One fused GRU application over all chunks/blocks.

                make_xe(c) -> list of DK x-side rhs APs [128, CH] (unused when
                gi_cache is given).
                h_in: list of DK [128, R] tiles (prev h), or None if first.
                hout(i) -> [128, R] output AP for block i.
                gi_cache: list of 3*DK [128, R] SBUF bf16 tiles with the
                precomputed x-side pre-activations (biases NOT included).
                """
                P = LP[li]
                DK = P["DK"]
                for c in range(NCH):
                    cc = slice(c * CH, (c + 1) * CH)
                    xe = None if gi_cache is not None else make_xe(c)

                    def xacc(ps, m, close):
                        """x-side of gate-block m into ps (opens the group)."""
                        if gi_cache is not None:
                            nc.tensor.matmul(ps[:], ident[:], gi_cache[m][:, cc],
                                             start=True, stop=close)
                        else:
                            for k in range(DK):
                                nc.tensor.matmul(ps[:], wcol(P, P["wih"], k, m),
                                                 xe[k], start=(k == 0),
                                                 stop=(k == DK - 1 and close))
                    for i in range(DK):
                        # --- r gate (m = i) ---
                        ps_r = psg.tile([128, CH], F32, tag="ps", name="ps_r")
                        xacc(ps_r, i, first)
                        if not first:
                            for k in range(DK):
                                nc.tensor.matmul(ps_r[:], wcol(P, P["whh"], k, i),
                                                 h_in[k][:, cc], start=False,
                                                 stop=(k == DK - 1))
                        r = rp.tile([128, CH], BF, tag="r", name="r_t")
                        nc.scalar.activation(r[:], ps_r[:], AF.Sigmoid,
                                             bias=P["brz"][:, i:i + 1])
                        # --- z gate (m = DK + i) ---
                        ps_z = psg.tile([128, CH], F32, tag="ps", name="ps_z")
                        xacc(ps_z, DK + i, first)
                        if not first:
                            for k in range(DK):
                                nc.tensor.matmul(ps_z[:], wcol(P, P["whh"], k, DK + i),
                                                 h_in[k][:, cc], start=False,
                                                 stop=(k == DK - 1))
                        z = zp.tile([128, CH], BF, tag="z", name="z_t")
                        nc.scalar.activation(z[:], ps_z[:], AF.Sigmoid,
                                             bias=P["brz"][:, DK + i:DK + i + 1])
                        # --- n gate: t = (gh_n + bhh_n) * r ---
                        t_ = tp.tile([128, CH], BF, tag="t", name="t_t")
                        if first:
                            nc.vector.tensor_scalar(t_[:], r[:],
                                                    P["bhhn"][:, i:i + 1], None,
                                                    op0=ALU.mult)
                        else:
                            ps_hn = psg.tile([128, CH], F32, tag="ps", name="ps_hn")
                            for k in range(DK):
                                nc.tensor.matmul(ps_hn[:], wcol(P, P["whh"], k, 2 * DK + i),
                                                 h_in[k][:, cc], start=(k == 0),
                                                 stop=(k == DK - 1))
                            nc.vector.scalar_tensor_tensor(
                                t_[:], ps_hn[:], P["bhhn"][:, i:i + 1], r[:],
                                op0=ALU.add, op1=ALU.mult)
                        # --- s = t + gi_n ; n = tanh(s + bih_n) ---
                        s_ = scp.tile([128, CH], BF, tag="s", name="s_t")
                        if gi_cache is not None:
                            nc.vector.tensor_add(s_[:], t_[:],
                                                 gi_cache[2 * DK + i][:, cc])
                        else:
                            ps_in = psg.tile([128, CH], F32, tag="ps", name="ps_in")
                            xacc(ps_in, 2 * DK + i, True)
                            nc.vector.tensor_add(s_[:], t_[:], ps_in[:])
                        n = np_p.tile([128, CH], BF, tag="n", name="n_t")
                        nc.scalar.activation(n[:], s_[:], AF.Tanh,
                                             bias=P["bihn"][:, i:i + 1])
                        # --- h' = n + z*(h-n)  (h=0 when first) ---
                        if first:
                            v = vp.tile([128, CH], BF, tag="v", name="v_t")
                            nc.vector.tensor_mul(v[:], n[:], z[:])
                            nc.vector.tensor_sub(hout(i)[:, cc], n[:], v[:])
                        else:
                            u = up.tile([128, CH], BF, tag="u", name="u_t")
                            nc.vector.tensor_sub(u[:], h_in[i][:, cc], n[:])
                            v = vp.tile([128, CH], BF, tag="v", name="v_t")
                            nc.vector.tensor_mul(v[:], u[:], z[:])
                            nc.vector.tensor_add(hout(i)[:, cc], n[:], v[:])

            def emit_enc_step(li, t, make_xe, h_in, gi_cache=None):
                P = LP[li]
                h_pool = h0p if li == 0 else h1p
                h_out = [h_pool.tile([128, R], BF, tag=f"h{li}", name=f"h{li}_{t}_{k}")
                         for k in range(P["DK"])]
                emit_gru(li, make_xe, h_in, lambda i: h_out[i][:],
                         first=(t == 0), gi_cache=gi_cache)
                return h_out

            def emit_l1_cache_fill(j):
                """Compute gi1[j][m] = Wih1 @ relu(emb(xs1 seg j)) into SBUF."""
                P = LP[1]
                make_xe = make_xe_embed(
                    1, lambda c, j=j: xs1[:, j * R + c * CH:j * R + (c + 1) * CH])
                for c in range(NCH):
                    cc = slice(c * CH, (c + 1) * CH)
                    xe = make_xe(c)
                    for m in range(3 * DK1):
                        ps = psg.tile([128, CH], F32, tag="ps", name="ps_gi")
                        for k in range(DK1):
                            nc.tensor.matmul(ps[:], wcol(P, P["wih"], k, m),
                                             xe[k], start=(k == 0),
                                             stop=(k == DK1 - 1))
                        nc.vector.tensor_copy(gi1[j][m][:, cc], ps[:])

            def emit_decoder(li, s_, h_fin):
                P = LP[li]
                DK, SG = P["DK"], P["SG"]
                hy = [hyp.tile([128, R], BF, tag="hy", name=f"hy{li}_{s_}_{i}")
                      for i in range(DK)]
                # full-R pos tiles; the 4x batch replication happens here via
                # 4 reads of the same unreplicated [128, ENC] DRAM slice.
                pts = []
                for k in range(DK):
                    pt = posp.tile([128, R], BF, tag="pos",
                                   name=f"pos{li}_{s_}_{k}")
                    for rep in range(BPC):
                        nc.sync.dma_start(pt[:, rep * ENC:(rep + 1) * ENC],
                                          posx_d[li][s_, k])
                    pts.append(pt)
                def make_xe(c):
                    return [pts[k][:, c * CH:(c + 1) * CH] for k in range(DK)]
                emit_gru(li, make_xe, h_fin, lambda i: hy[i][:], first=False)
                for c in range(NCH):
                    cc = slice(c * CH, (c + 1) * CH)
                    ps = psy.tile([SG, CH], F32, tag="psy", name="ps_y")
                    for k in range(DK):
                        nc.tensor.matmul(ps[:], P["wpred"][:, k * SG:(k + 1) * SG],
                                         hy[k][:, cc], start=(k == 0),
                                         stop=(k == DK - 1))
                    y = yp.tile([SG, CH], F32, tag="y", name="y_t")
                    nc.scalar.activation(y[:], ps[:], AF.Identity,
                                         bias=P["bpred"][0:SG, 0:1])
                    nc.sync.dma_start(y_d[li][s_, :, cc], y[:])

            # ---------------- encoder ----------------
            h0 = None
            h1 = None
            t1 = 0
            for t in range(l0_steps):
                xs_t = x0p.tile([SG0, R], BF, tag="xs0", name=f"xs0_{t}")
                nc.sync.dma_start(xs_t[:], xseg0_d[t])
                h0 = emit_enc_step(
                    0, t, make_xe_embed(0, lambda c, xs_t=xs_t: xs_t[:, c * CH:(c + 1) * CH]),
                    h0)
                for _ in range(4):
                    if t1 < l1_steps:
                        j = t1 % 4
                        if t1 < n_var:
                            emit_l1_cache_fill(j)
                        h1 = emit_enc_step(1, t1, None, h1, gi_cache=gi1[j])
                        t1 += 1
            while t1 < l1_steps:
                j = t1 % 4
                if t1 < n_var:
                    emit_l1_cache_fill(j)
                h1 = emit_enc_step(1, t1, None, h1, gi_cache=gi1[j])
                t1 += 1

            # ---------------- decoders ----------------
            emit_decoder(0, 0, h0)
            emit_decoder(1, 0, h1)
            emit_decoder(0, 1, h0)
            emit_decoder(1, 1, h1)
            emit_decoder(1, 2, h1)
            emit_decoder(1, 3, h1)

    nc.compile()
    return nc


def get_nc(l0_steps=T0, l1_steps=T1):
    key = (l0_steps, l1_steps)
    if key not in _CACHE:
        _CACHE[key] = _build_nc(l0_steps, l1_steps)
    return _CACHE[key]


# ==================== host side ====================

BF_NP = mybir.dt.np(mybir.dt.bfloat16)


def _prep_shared(inp):
    f = np.float32
    m = {}
    for li, d in ((0, D0), (1, D1)):
        dk = (DK0, DK1)[li]
        sg = (SG0, SG1)[li]
        m[f"wembT{li}"] = np.ascontiguousarray(inp[f"W_emb{li}"].T).astype(BF_NP)
        m[f"wihT{li}"] = np.ascontiguousarray(
            inp[f"Wih{li}"].T.reshape(dk, 128, 3 * d)).astype(BF_NP)
        m[f"whhT{li}"] = np.ascontiguousarray(
            inp[f"Whh{li}"].T.reshape(dk, 128, 3 * d)).astype(BF_NP)
        m[f"wpredT{li}"] = np.ascontiguousarray(
            inp[f"Wpred{li}"].T.reshape(dk, 128, sg)).astype(BF_NP)
        bih, bhh = inp[f"bih{li}"].astype(f), inp[f"bhh{li}"].astype(f)
        m[f"brz{li}"] = np.ascontiguousarray(
            (bih + bhh)[:2 * d].reshape(2 * dk, 128).T)
        m[f"bihn{li}"] = np.ascontiguousarray(bih[2 * d:].reshape(dk, 128).T)
        m[f"bhhn{li}"] = np.ascontiguousarray(bhh[2 * d:].reshape(dk, 128).T)
        m[f"bemb{li}"] = np.ascontiguousarray(
            inp[f"b_emb{li}"].astype(f).reshape(dk, 128).T)
        bp = np.zeros((128, 1), f)
        bp[:sg, 0] = inp[f"bpred{li}"].astype(f)
        m[f"bpred{li}"] = bp
        half = d // 2
        pos, chan = inp[f"pos{li}"].astype(f), inp[f"chan{li}"].astype(f)
        S = pos.shape[0]
        base = np.concatenate(
            [np.broadcast_to(pos[:, None, :], (S, ENC, half)),
             np.broadcast_to(chan[None, :, :], (S, ENC, half))], axis=-1)
        posx = base.transpose(0, 2, 1)                        # [S, d, ENC]
        m[f"posx{li}"] = np.ascontiguousarray(
            posx.reshape(S, dk, 128, ENC)).astype(BF_NP)
    m["ident"] = np.eye(128, dtype=BF_NP)
    return m


def _prep_core(x, c):
    f = np.float32
    xb = x[BPC * c:BPC * (c + 1)].astype(f)
    last = xb[:, -1:, :]
    xc = (xb - last).transpose(0, 2, 1).reshape(R, SEQ)
    xseg0 = np.ascontiguousarray(
        xc.reshape(R, T0, SG0).transpose(1, 2, 0)).astype(BF_NP)
    xseg1 = np.ascontiguousarray(
        xc[:, :4 * SG1].reshape(R, 4, SG1).transpose(2, 1, 0).reshape(SG1, 4 * R)
    ).astype(BF_NP)
    return xseg0, xseg1


def make_in_maps(inp):
    """Build per-core packed input maps ({'bb': .., 'bf': ..})."""
    x = np.asarray(inp["x"], np.float32)
    shared = _prep_shared({k: np.asarray(v) for k, v in inp.items()})
    bf = np.empty(N_F32, np.float32)
    for name, shape in PACK_F32:
        off, _ = OFF_F32[name]
        bf[off:off + int(np.prod(shape))] = shared[name].ravel()
    bb_tail = np.empty(N_BF, BF_NP)
    for name, shape in PACK_BF:
        if name in ("xseg0", "xseg1"):
            continue
        off, _ = OFF_BF[name]
        bb_tail[off:off + int(np.prod(shape))] = shared[name].ravel()
    in_maps = []
    for c in range(NCORE):
        xseg0, xseg1 = _prep_core(x, c)
        bb = bb_tail.copy()
        o0, _ = OFF_BF["xseg0"]
        bb[o0:o0 + xseg0.size] = xseg0.ravel()
        o1, _ = OFF_BF["xseg1"]
        bb[o1:o1 + xseg1.size] = xseg1.ravel()
        in_maps.append({"bb": bb, "bf": bf})
    return in_maps


def split_y(yy_core):
    """Split one core's packed output into (y0, y1)."""
    y0 = yy_core[:NY0].reshape(S0, SG0, R)
    y1 = yy_core[NY0:NY0 + NY1].reshape(S1, SG1, R)
    return y0, y1


def assemble_output(yy_per_core, x):
    """yy_per_core: list of 8 flat yy arrays -> full [B, PRED, ENC] output."""
    ys = [split_y(np.asarray(yy).ravel()) for yy in yy_per_core]
    full0 = np.concatenate([y0 for y0, _ in ys], axis=2)
    full1 = np.concatenate([y1 for _, y1 in ys], axis=2)
    # out[b, s_*seg+j, e] = y[s_, j, n=(b,e)]
    yl0 = full0.reshape(S0, SG0, B, ENC).transpose(2, 0, 1, 3).reshape(B, PRED, ENC)
    yl1 = full1.reshape(S1, SG1, B, ENC).transpose(2, 0, 1, 3).reshape(B, PRED, ENC)
    return ((yl0 + yl1) / 2.0 + x[:, -1:, :]).astype(np.float32)


def kernel(**inputs):
    x = np.asarray(inputs["x"], np.float32)
    in_maps = make_in_maps(inputs)
    nc = get_nc()
    res = run_bass_kernel_spmd(nc, in_maps, list(range(NCORE))).results
    return assemble_output([res[c]["yy"] for c in range(NCORE)], x)


# revision 3
# speedup vs baseline: 1.6636x; 1.6636x over previous
"""Trainium2 Bass kernel for nn_Hierarch_RNN (hierarchical 2-layer GRU), v2.

Changes vs v1 baseline:
  - bf16 for all matmul operands + elementwise state (PSUM stays f32):
    2x DVE throughput on SBUF tensor-tensor ops, halved SBUF/DMA traffic,
    same PE rate as f32r.
  - Layer-1 input is periodic with period 4 (segments 0..3 repeat 15x),
    so the x-side gate pre-activations gi = Wih1 @ relu(emb) are computed
    once per unique segment and cached in SBUF; the 56 repeated steps
    initialize the r/z PSUM with an identity matmul from the cache and the
    n-gate folds the cached term into the existing DVE add. Saves 30 of 78
    matmuls per L1 step and all embed work.
  - Embed relu moved from ScalarE to a fused DVE tensor_scalar
    (add-bias, max-0) to balance engine load (ScalarE was near-critical).
  - L1 tanh batched over the full row dim (one act per 128-block instead
    of three) to amortize the ~350-cycle ACT instruction overhead.
"""
import numpy as np

import concourse.mybir as mybir
import concourse.tile as tile
from concourse import bacc
from concourse.bass_utils import run_bass_kernel_spmd

F32 = mybir.dt.float32
BF = mybir.dt.bfloat16
AF = mybir.ActivationFunctionType
ALU = mybir.AluOpType

B, SEQ, PRED, ENC = 32, 720, 96, 321
NCORE, BPC = 8, 4
R = BPC * ENC                 # 1284 rows per core
CH, NCH = 428, 3              # row chunks (428 f32 <= 512/bank)
# layer params: d, seg_len, n 128-blocks of d (DK), decoder steps S
D0, SG0, DK0, S0, T0 = 512, 48, 4, 2, 15
D1, SG1, DK1, S1, T1 = 256, 24, 2, 4, 60

_CACHE = {}

# Packed-input layouts: all bf16 tensors in one flat blob, all f32 in
# another — per-argument dispatch overhead through the exec path is large,
# so the kernel takes 2 inputs + 1 output instead of ~31/2.
# posx ships unreplicated ([.., 321] instead of [.., R=4*321]); the 4x
# batch replication happens via 4 on-device DMA reads.
PACK_BF = [
    ("xseg0", (T0, SG0, R)),
    ("xseg1", (SG1, 4 * R)),
    ("wihT0", (DK0, 128, 3 * D0)), ("wihT1", (DK1, 128, 3 * D1)),
    ("whhT0", (DK0, 128, 3 * D0)), ("whhT1", (DK1, 128, 3 * D1)),
    ("wembT0", (SG0, D0)), ("wembT1", (SG1, D1)),
    ("wpredT0", (DK0, 128, SG0)), ("wpredT1", (DK1, 128, SG1)),
    ("posx0", (S0, DK0, 128, ENC)), ("posx1", (S1, DK1, 128, ENC)),
    ("ident", (128, 128)),
]
PACK_F32 = [
    ("brz0", (128, 2 * DK0)), ("brz1", (128, 2 * DK1)),
    ("bihn0", (128, DK0)), ("bihn1", (128, DK1)),
    ("bhhn0", (128, DK0)), ("bhhn1", (128, DK1)),
    ("bemb0", (128, DK0)), ("bemb1", (128, DK1)),
    ("bpred0", (128, 1)), ("bpred1", (128, 1)),
]


def _offsets(spec):
    out, off = {}, 0
    for name, shape in spec:
        n = int(np.prod(shape))
        out[name] = (off, shape)
        off += n
    return out, off


OFF_BF, N_BF = _offsets(PACK_BF)
OFF_F32, N_F32 = _offsets(PACK_F32)
NY0, NY1 = S0 * SG0 * R, S1 * SG1 * R


def _build_nc(l0_steps=T0, l1_steps=T1):
    nc = bacc.Bacc("TRN2", target_bir_lowering=False, debug=False,
                   num_devices=NCORE)

    # ---------------- DRAM tensors (packed) ----------------
    bb_d = nc.dram_tensor("bb", [N_BF], BF, kind="ExternalInput")
    bf_d = nc.dram_tensor("bf", [N_F32], F32, kind="ExternalInput")
    yy_d = nc.dram_tensor("yy", [NY0 + NY1], F32, kind="ExternalOutput")

    def vbf(name):
        off, shape = OFF_BF[name]
        n = int(np.prod(shape))
        ap = bb_d[off:off + n]
        if len(shape) == 2:
            return ap.rearrange("(a b) -> a b", a=shape[0])
        if len(shape) == 3:
            return ap.rearrange("(a b c) -> a b c", a=shape[0], b=shape[1])
        return ap.rearrange("(a b c d) -> a b c d",
                            a=shape[0], b=shape[1], c=shape[2])

    def vf32(name):
        off, shape = OFF_F32[name]
        n = int(np.prod(shape))
        return bf_d[off:off + n].rearrange("(a b) -> a b", a=shape[0])

    xseg0_d = vbf("xseg0")
    xseg1_d = vbf("xseg1")
    wih_d = [vbf("wihT0"), vbf("wihT1")]
    whh_d = [vbf("whhT0"), vbf("whhT1")]
    wemb_d = [vbf("wembT0"), vbf("wembT1")]
    wpred_d = [vbf("wpredT0"), vbf("wpredT1")]
    brz_d = [vf32("brz0"), vf32("brz1")]
    bihn_d = [vf32("bihn0"), vf32("bihn1")]
    bhhn_d = [vf32("bhhn0"), vf32("bhhn1")]
    bemb_d = [vf32("bemb0"), vf32("bemb1")]
    bpred_d = [vf32("bpred0"), vf32("bpred1")]
    posx_d = [vbf("posx0"), vbf("posx1")]
    ident_d = vbf("ident")
    y_d = [yy_d[0:NY0].rearrange("(a b c) -> a b c", a=S0, b=SG0),
           yy_d[NY0:NY0 + NY1].rearrange("(a b c) -> a b c", a=S1, b=SG1)]

    with tile.TileContext(nc) as tc:
        with tc.tile_pool(name="const", bufs=1) as cp, \
             tc.tile_pool(name="x0p", bufs=2) as x0p, \
             tc.tile_pool(name="xep", bufs=6) as xep, \
             tc.tile_pool(name="h0p", bufs=8) as h0p, \
             tc.tile_pool(name="h1p", bufs=4) as h1p, \
             tc.tile_pool(name="posp", bufs=5) as posp, \
             tc.tile_pool(name="hyp", bufs=6) as hyp, \
             tc.tile_pool(name="rp", bufs=4) as rp, \
             tc.tile_pool(name="zp", bufs=4) as zp, \
             tc.tile_pool(name="np_", bufs=4) as np_p, \
             tc.tile_pool(name="scp", bufs=4) as scp, \
             tc.tile_pool(name="tp", bufs=4) as tp, \
             tc.tile_pool(name="up", bufs=4) as up, \
             tc.tile_pool(name="vp", bufs=4) as vp, \
             tc.tile_pool(name="yp", bufs=2) as yp, \
             tc.tile_pool(name="psg", bufs=6, space="PSUM") as psg, \
             tc.tile_pool(name="psy", bufs=2, space="PSUM") as psy:

            # ---------------- load constants ----------------
            def load_w(dram, k_tiles, cols, nm):
                t = cp.tile([128, k_tiles * cols], BF, tag=f"c_{nm}",
                            name=f"c_{nm}")
                for k in range(k_tiles):
                    nc.sync.dma_start(t[:, k * cols:(k + 1) * cols], dram[k])
                return t

            wih_sb = [load_w(wih_d[0], DK0, 3 * D0, "wih0"),
                      load_w(wih_d[1], DK1, 3 * D1, "wih1")]
            whh_sb = [load_w(whh_d[0], DK0, 3 * D0, "whh0"),
                      load_w(whh_d[1], DK1, 3 * D1, "whh1")]
            wpred_sb = [load_w(wpred_d[0], DK0, SG0, "wpred0"),
                        load_w(wpred_d[1], DK1, SG1, "wpred1")]
            wemb_sb = []
            for li, (sg, d) in enumerate(((SG0, D0), (SG1, D1))):
                t = cp.tile([sg, d], BF, tag=f"c_wemb{li}", name=f"c_wemb{li}")
                nc.sync.dma_start(t[:], wemb_d[li][:])
                wemb_sb.append(t)
            def load_b(dram, cols, nm):
                t = cp.tile([128, cols], F32, tag=f"c_{nm}", name=f"c_{nm}")
                nc.sync.dma_start(t[:], dram[:])
                return t
            brz_sb = [load_b(brz_d[0], 2 * DK0, "brz0"), load_b(brz_d[1], 2 * DK1, "brz1")]
            bihn_sb = [load_b(bihn_d[0], DK0, "bihn0"), load_b(bihn_d[1], DK1, "bihn1")]
            bhhn_sb = [load_b(bhhn_d[0], DK0, "bhhn0"), load_b(bhhn_d[1], DK1, "bhhn1")]
            bemb_sb = [load_b(bemb_d[0], DK0, "bemb0"), load_b(bemb_d[1], DK1, "bemb1")]
            bpred_sb = [load_b(bpred_d[0], 1, "bpred0"), load_b(bpred_d[1], 1, "bpred1")]
            xs1 = cp.tile([SG1, 4 * R], BF, tag="c_xs1", name="c_xs1")
            nc.sync.dma_start(xs1[:], xseg1_d[:])
            ident = cp.tile([128, 128], BF, tag="c_ident", name="c_ident")
            nc.sync.dma_start(ident[:], ident_d[:])
            # L1 x-side gate cache: gi1[j][m] = (Wih1 @ relu(emb(xs1_j)))[m]
            # for the 4 unique input segments, m over the 6 output 128-blocks.
            n_var = min(4, l1_steps)
            gi1 = [[cp.tile([128, R], BF, tag=f"c_gi1_{j}_{m}",
                            name=f"c_gi1_{j}_{m}")
                    for m in range(3 * DK1)] for j in range(n_var)]

            LP = [dict(D=D0, DK=DK0, SG=SG0, wih=wih_sb[0], whh=whh_sb[0],
                       wemb=wemb_sb[0], wpred=wpred_sb[0], brz=brz_sb[0],
                       bihn=bihn_sb[0], bhhn=bhhn_sb[0], bemb=bemb_sb[0],
                       bpred=bpred_sb[0]),
                  dict(D=D1, DK=DK1, SG=SG1, wih=wih_sb[1], whh=whh_sb[1],
                       wemb=wemb_sb[1], wpred=wpred_sb[1], brz=brz_sb[1],
                       bihn=bihn_sb[1], bhhn=bhhn_sb[1], bemb=bemb_sb[1],
                       bpred=bpred_sb[1])]

            def wcol(P, wt, k, m):
                """AP of [128,128] weight block: k-tile k, m-tile m of 3d."""
                c0 = k * 3 * P["D"] + m * 128
                return wt[:, c0:c0 + 128]

            def make_xe_embed(li, xsrc_fn):
                """Returns make_xe(c): emits per-chunk embed, returns DK APs."""
                P = LP[li]
                def make_xe(c):
                    aps = []
                    for k in range(P["DK"]):
                        ps = psg.tile([128, CH], F32, tag="ps", name="ps_e")
                        nc.tensor.matmul(ps[:], P["wemb"][:, k * 128:(k + 1) * 128],
                                         xsrc_fn(c), start=True, stop=True)
                        xe = xep.tile([128, CH], BF, tag="xe", name=f"xe{li}_{k}")
                        nc.scalar.activation(xe[:], ps[:], AF.Relu,
                                             bias=P["bemb"][:, k:k + 1])
                        aps.append(xe[:])
                    return aps
                return make_xe

            def emit_gru(li, make_xe, h_in, hout, first, gi_cache=None):
                """One fused GRU application over all chunks/blocks.

                make_xe(c) -> list of DK x-side rhs APs [128, CH] (unused when
                gi_cache is given).
                h_in: list of DK [128, R] tiles (prev h), or None if first.
                hout(i) -> [128, R] output AP for block i.
                gi_cache: list of 3*DK [128, R] SBUF bf16 tiles with the
                precomputed x-side pre-activations (biases NOT included).
                """
                P = LP[li]
                DK = P["DK"]
                for c in range(NCH):
                    cc = slice(c * CH, (c + 1) * CH)
                    xe = None if gi_cache is not None else make_xe(c)

                    def xacc(ps, m, close):
                        """x-side of gate-block m into ps (opens the group)."""
                        if gi_cache is not None:
                            nc.tensor.matmul(ps[:], ident[:], gi_cache[m][:, cc],
                                             start=True, stop=close)
                        else:
                            for k in range(DK):
                                nc.tensor.matmul(ps[:], wcol(P, P["wih"], k, m),
                                                 xe[k], start=(k == 0),
                                                 stop=(k == DK - 1 and close))
                    for i in range(DK):
                        # --- r gate (m = i) ---
                        ps_r = psg.tile([128, CH], F32, tag="ps", name="ps_r")
                        xacc(ps_r, i, first)
                        if not first:
                            for k in range(DK):
                                nc.tensor.matmul(ps_r[:], wcol(P, P["whh"], k, i),
                                                 h_in[k][:, cc], start=False,
                                                 stop=(k == DK - 1))
                        r = rp.tile([128, CH], BF, tag="r", name="r_t")
                        nc.scalar.activation(r[:], ps_r[:], AF.Sigmoid,
                                             bias=P["brz"][:, i:i + 1])
                        # --- z gate (m = DK + i) ---
                        ps_z = psg.tile([128, CH], F32, tag="ps", name="ps_z")
                        xacc(ps_z, DK + i, first)
                        if not first:
                            for k in range(DK):
                                nc.tensor.matmul(ps_z[:], wcol(P, P["whh"], k, DK + i),
                                                 h_in[k][:, cc], start=False,
                                                 stop=(k == DK - 1))
                        z = zp.tile([128, CH], BF, tag="z", name="z_t")
                        nc.scalar.activation(z[:], ps_z[:], AF.Sigmoid,
                                             bias=P["brz"][:, DK + i:DK + i + 1])
                        # --- n gate: t = (gh_n + bhh_n) * r ---
                        t_ = tp.tile([128, CH], BF, tag="t", name="t_t")
                        if first:
                            nc.vector.tensor_scalar(t_[:], r[:],
                                                    P["bhhn"][:, i:i + 1], None,
                                                    op0=ALU.mult)
                        else:
                            ps_hn = psg.tile([128, CH], F32, tag="ps", name="ps_hn")
                            for k in range(DK):
                                nc.tensor.matmul(ps_hn[:], wcol(P, P["whh"], k, 2 * DK + i),
                                                 h_in[k][:, cc], start=(k == 0),
                                                 stop=(k == DK - 1))
                            nc.vector.scalar_tensor_tensor(
                                t_[:], ps_hn[:], P["bhhn"][:, i:i + 1], r[:],
                                op0=ALU.add, op1=ALU.mult)
                        # --- s = t + gi_n ; n = tanh(s + bih_n) ---
                        s_ = scp.tile([128, CH], BF, tag="s", name="s_t")
                        if gi_cache is not None:
                            nc.vector.tensor_add(s_[:], t_[:],
                                                 gi_cache[2 * DK + i][:, cc])
                        else:
                            ps_in = psg.tile([128, CH], F32, tag="ps", name="ps_in")
                            xacc(ps_in, 2 * DK + i, True)
                            nc.vector.tensor_add(s_[:], t_[:], ps_in[:])
                        n = np_p.tile([128, CH], BF, tag="n", name="n_t")
                        nc.scalar.activation(n[:], s_[:], AF.Tanh,
                                             bias=P["bihn"][:, i:i + 1])
                        # --- h' = n + z*(h-n)  (h=0 when first) ---
                        if first:
                            v = vp.tile([128, CH], BF, tag="v", name="v_t")
                            nc.vector.tensor_mul(v[:], n[:], z[:])
                            nc.vector.tensor_sub(hout(i)[:, cc], n[:], v[:])
                        else:
                            u = up.tile([128, CH], BF, tag="u", name="u_t")
                            nc.gpsimd.tensor_sub(u[:], h_in[i][:, cc], n[:])
                            v = vp.tile([128, CH], BF, tag="v", name="v_t")
                            nc.gpsimd.tensor_mul(v[:], u[:], z[:])
                            nc.vector.tensor_add(hout(i)[:, cc], n[:], v[:])

            def emit_enc_step(li, t, make_xe, h_in, gi_cache=None):
                P = LP[li]
                h_pool = h0p if li == 0 else h1p
                h_out = [h_pool.tile([128, R], BF, tag=f"h{li}", name=f"h{li}_{t}_{k}")
                         for k in range(P["DK"])]
                emit_gru(li, make_xe, h_in, lambda i: h_out[i][:],
                         first=(t == 0), gi_cache=gi_cache)
                return h_out

            def emit_l1_cache_fill(j):
                """Compute gi1[j][m] = Wih1 @ relu(emb(xs1 seg j)) into SBUF."""
                P = LP[1]
                make_xe = make_xe_embed(
                    1, lambda c, j=j: xs1[:, j * R + c * CH:j * R + (c + 1) * CH])
                for c in range(NCH):
                    cc = slice(c * CH, (c + 1) * CH)
                    xe = make_xe(c)
                    for m in range(3 * DK1):
                        ps = psg.tile([128, CH], F32, tag="ps", name="ps_gi")
                        for k in range(DK1):
                            nc.tensor.matmul(ps[:], wcol(P, P["wih"], k, m),
                                             xe[k], start=(k == 0),
                                             stop=(k == DK1 - 1))
                        nc.vector.tensor_copy(gi1[j][m][:, cc], ps[:])

            def emit_decoder(li, s_, h_fin):
                P = LP[li]
                DK, SG = P["DK"], P["SG"]
                hy = [hyp.tile([128, R], BF, tag="hy", name=f"hy{li}_{s_}_{i}")
                      for i in range(DK)]
                # full-R pos tiles; the 4x batch replication happens here via
                # 4 reads of the same unreplicated [128, ENC] DRAM slice.
                pts = []
                for k in range(DK):
                    pt = posp.tile([128, R], BF, tag="pos",
                                   name=f"pos{li}_{s_}_{k}")
                    for rep in range(BPC):
                        nc.sync.dma_start(pt[:, rep * ENC:(rep + 1) * ENC],
                                          posx_d[li][s_, k])
                    pts.append(pt)
                def make_xe(c):
                    return [pts[k][:, c * CH:(c + 1) * CH] for k in range(DK)]
                emit_gru(li, make_xe, h_fin, lambda i: hy[i][:], first=False)
                for c in range(NCH):
                    cc = slice(c * CH, (c + 1) * CH)
                    ps = psy.tile([SG, CH], F32, tag="psy", name="ps_y")
                    for k in range(DK):
                        nc.tensor.matmul(ps[:], P["wpred"][:, k * SG:(k + 1) * SG],
                                         hy[k][:, cc], start=(k == 0),
                                         stop=(k == DK - 1))
                    y = yp.tile([SG, CH], F32, tag="y", name="y_t")
                    nc.scalar.activation(y[:], ps[:], AF.Identity,
                                         bias=P["bpred"][0:SG, 0:1])
                    nc.sync.dma_start(y_d[li][s_, :, cc], y[:])

            # ---------------- encoder ----------------
            h0 = None
            h1 = None
            t1 = 0
            for t in range(l0_steps):
                xs_t = x0p.tile([SG0, R], BF, tag="xs0", name=f"xs0_{t}")
                nc.sync.dma_start(xs_t[:], xseg0_d[t])
                h0 = emit_enc_step(
                    0, t, make_xe_embed(0, lambda c, xs_t=xs_t: xs_t[:, c * CH:(c + 1) * CH]),
                    h0)
                for _ in range(4):
                    if t1 < l1_steps:
                        j = t1 % 4
                        if t1 < n_var:
                            emit_l1_cache_fill(j)
                        h1 = emit_enc_step(1, t1, None, h1, gi_cache=gi1[j])
                        t1 += 1
            while t1 < l1_steps:
                j = t1 % 4
                if t1 < n_var:
                    emit_l1_cache_fill(j)
                h1 = emit_enc_step(1, t1, None, h1, gi_cache=gi1[j])
                t1 += 1

            # ---------------- decoders ----------------
            emit_decoder(0, 0, h0)
            emit_decoder(1, 0, h1)
            emit_decoder(0, 1, h0)
            emit_decoder(1, 1, h1)
            emit_decoder(1, 2, h1)
            emit_decoder(1, 3, h1)

    nc.compile()
    return nc


def get_nc(l0_steps=T0, l1_steps=T1):
    key = (l0_steps, l1_steps)
    if key not in _CACHE:
        _CACHE[key] = _build_nc(l0_steps, l1_steps)
    return _CACHE[key]


# ==================== host side ====================

BF_NP = mybir.dt.np(mybir.dt.bfloat16)


def _prep_shared(inp):
    f = np.float32
    m = {}
    for li, d in ((0, D0), (1, D1)):
        dk = (DK0, DK1)[li]
        sg = (SG0, SG1)[li]
        m[f"wembT{li}"] = np.ascontiguousarray(inp[f"W_emb{li}"].T).astype(BF_NP)
        m[f"wihT{li}"] = np.ascontiguousarray(
            inp[f"Wih{li}"].T.reshape(dk, 128, 3 * d)).astype(BF_NP)
        m[f"whhT{li}"] = np.ascontiguousarray(
            inp[f"Whh{li}"].T.reshape(dk, 128, 3 * d)).astype(BF_NP)
        m[f"wpredT{li}"] = np.ascontiguousarray(
            inp[f"Wpred{li}"].T.reshape(dk, 128, sg)).astype(BF_NP)
        bih, bhh = inp[f"bih{li}"].astype(f), inp[f"bhh{li}"].astype(f)
        m[f"brz{li}"] = np.ascontiguousarray(
            (bih + bhh)[:2 * d].reshape(2 * dk, 128).T)
        m[f"bihn{li}"] = np.ascontiguousarray(bih[2 * d:].reshape(dk, 128).T)
        m[f"bhhn{li}"] = np.ascontiguousarray(bhh[2 * d:].reshape(dk, 128).T)
        m[f"bemb{li}"] = np.ascontiguousarray(
            inp[f"b_emb{li}"].astype(f).reshape(dk, 128).T)
        bp = np.zeros((128, 1), f)
        bp[:sg, 0] = inp[f"bpred{li}"].astype(f)
        m[f"bpred{li}"] = bp
        half = d // 2
        pos, chan = inp[f"pos{li}"].astype(f), inp[f"chan{li}"].astype(f)
        S = pos.shape[0]
        base = np.concatenate(
            [np.broadcast_to(pos[:, None, :], (S, ENC, half)),
             np.broadcast_to(chan[None, :, :], (S, ENC, half))], axis=-1)
        posx = base.transpose(0, 2, 1)                        # [S, d, ENC]
        m[f"posx{li}"] = np.ascontiguousarray(
            posx.reshape(S, dk, 128, ENC)).astype(BF_NP)
    m["ident"] = np.eye(128, dtype=BF_NP)
    return m


def _prep_core(x, c):
    f = np.float32
    xb = x[BPC * c:BPC * (c + 1)].astype(f)
    last = xb[:, -1:, :]
    xc = (xb - last).transpose(0, 2, 1).reshape(R, SEQ)
    xseg0 = np.ascontiguousarray(
        xc.reshape(R, T0, SG0).transpose(1, 2, 0)).astype(BF_NP)
    xseg1 = np.ascontiguousarray(
        xc[:, :4 * SG1].reshape(R, 4, SG1).transpose(2, 1, 0).reshape(SG1, 4 * R)
    ).astype(BF_NP)
    return xseg0, xseg1


def make_in_maps(inp):
    """Build per-core packed input maps ({'bb': .., 'bf': ..})."""
    x = np.asarray(inp["x"], np.float32)
    shared = _prep_shared({k: np.asarray(v) for k, v in inp.items()})
    bf = np.empty(N_F32, np.float32)
    for name, shape in PACK_F32:
        off, _ = OFF_F32[name]
        bf[off:off + int(np.prod(shape))] = shared[name].ravel()
    bb_tail = np.empty(N_BF, BF_NP)
    for name, shape in PACK_BF:
        if name in ("xseg0", "xseg1"):
            continue
        off, _ = OFF_BF[name]
        bb_tail[off:off + int(np.prod(shape))] = shared[name].ravel()
    in_maps = []
    for c in range(NCORE):
        xseg0, xseg1 = _prep_core(x, c)
        bb = bb_tail.copy()
        o0, _ = OFF_BF["xseg0"]
        bb[o0:o0 + xseg0.size] = xseg0.ravel()
        o1, _ = OFF_BF["xseg1"]
        bb[o1:o1 + xseg1.size] = xseg1.ravel()
        in_maps.append({"bb": bb, "bf": bf})
    return in_maps


def split_y(yy_core):
    """Split one core's packed output into (y0, y1)."""
    y0 = yy_core[:NY0].reshape(S0, SG0, R)
    y1 = yy_core[NY0:NY0 + NY1].reshape(S1, SG1, R)
    return y0, y1


def assemble_output(yy_per_core, x):
    """yy_per_core: list of 8 flat yy arrays -> full [B, PRED, ENC] output."""
    ys = [split_y(np.asarray(yy).ravel()) for yy in yy_per_core]
    full0 = np.concatenate([y0 for y0, _ in ys], axis=2)
    full1 = np.concatenate([y1 for _, y1 in ys], axis=2)
    # out[b, s_*seg+j, e] = y[s_, j, n=(b,e)]
    yl0 = full0.reshape(S0, SG0, B, ENC).transpose(2, 0, 1, 3).reshape(B, PRED, ENC)
    yl1 = full1.reshape(S1, SG1, B, ENC).transpose(2, 0, 1, 3).reshape(B, PRED, ENC)
    return ((yl0 + yl1) / 2.0 + x[:, -1:, :]).astype(np.float32)


def kernel(**inputs):
    x = np.asarray(inputs["x"], np.float32)
    in_maps = make_in_maps(inputs)
    nc = get_nc()
    res = run_bass_kernel_spmd(nc, in_maps, list(range(NCORE))).results
    return assemble_output([res[c]["yy"] for c in range(NCORE)], x)


# revision 4
# speedup vs baseline: 2.1763x; 1.3081x over previous
"""Trainium2 Bass kernel for nn_Hierarch_RNN (hierarchical 2-layer GRU).

Data-parallel over batch (32 batches -> 4/core on 8 cores); on-chip layout
is feature-major [d, rows] with rows-per-core R=1284 split into 3 chunks of
428 (one PSUM bank each). Key optimizations vs the f32r baseline
(CoreSim cost model: 1984us -> 1332us):
  - bf16 matmuls/elementwise state (PSUM stays f32); fp8(e4m3) DoubleRow
    matmuls for all x-side gate projections (Wih @ xe, Wih @ pos): two
    contraction k-tiles per instruction at ~2x throughput. Scales are
    folded so every gate PSUM is uniformly *8192 (fp8 weights *256,
    fp8 activations *32, bf16 Whh *8192) and the sigmoid/tanh activations
    descale for free via their scale= operand. End-to-end rel err 1.7e-3.
  - Layer-1 input is periodic with period 4 (segments 0..3 repeat 15x), so
    x-side pre-activations are computed once per unique segment, cached in
    SBUF, and re-injected into PSUM with an identity matmul (r/z) or a DVE
    add (n). Saves 30 of 78 matmuls per L1 step plus all embed work.
  - L0 and L1 steps interleaved 1:4 so the tensor engine always has
    independent work across each GRU step's sequential tail.
  - h-update (h-n)*z chain partially offloaded to GPSIMD; per-instruction
    overheads amortized chunk-wise; decoder pos embeddings ship
    unreplicated and are broadcast 4x by on-device DMA.
  - All inputs packed into 3 flat blobs (bf16/fp8/f32) + 1 output blob:
    per-argument dispatch overhead through the PJRT execute path dwarfs
    the NEFF time at ~30 arguments.
"""
import numpy as np

import concourse.mybir as mybir
import concourse.tile as tile
from concourse import bacc
from concourse.bass_utils import run_bass_kernel_spmd

F32 = mybir.dt.float32
BF = mybir.dt.bfloat16
F8 = mybir.dt.float8e4
AF = mybir.ActivationFunctionType
ALU = mybir.AluOpType
DR = mybir.MatmulPerfMode.DoubleRow

# fp8 scale folding: Wih ships as fp8 * WS, xe/pos as fp8 * XS (XS folded
# into Wemb/bemb host-side), Whh ships as bf16 * WS*XS, so every gate PSUM
# is uniformly scaled by S = WS*XS and the activations descale via scale=.
WS, XS = 256.0, 32.0
S_SC = WS * XS
SINV = 1.0 / S_SC
RPAD = 1296               # %16-aligned j-stride for DoubleRow rhs (>= R)
CHPAD = 432               # %16-aligned j-stride for xe pair tiles (>= CH)

B, SEQ, PRED, ENC = 32, 720, 96, 321
NCORE, BPC = 8, 4
R = BPC * ENC                 # 1284 rows per core
CH, NCH = 428, 3              # row chunks (428 f32 <= 512/bank)
# layer params: d, seg_len, n 128-blocks of d (DK), decoder steps S
D0, SG0, DK0, S0, T0 = 512, 48, 4, 2, 15
D1, SG1, DK1, S1, T1 = 256, 24, 2, 4, 60

_CACHE = {}

# Packed-input layouts: all bf16 tensors in one flat blob, all f32 in
# another — per-argument dispatch overhead through the exec path is large,
# so the kernel takes 2 inputs + 1 output instead of ~31/2.
# posx ships unreplicated ([.., 321] instead of [.., R=4*321]); the 4x
# batch replication happens via 4 on-device DMA reads.
PACK_BF = [
    ("xseg0", (T0, SG0, R)),
    ("xseg1", (SG1, 4 * R)),
    ("whhT0", (DK0, 128, 3 * D0)), ("whhT1", (DK1, 128, 3 * D1)),
    ("wembT0", (SG0, D0)), ("wembT1", (SG1, D1)),
    ("wpredT0", (DK0, 128, SG0)), ("wpredT1", (DK1, 128, SG1)),
    ("ident", (128, 128)),
]
PACK_F8 = [
    ("wihT0", (DK0, 128, 3 * D0)), ("wihT1", (DK1, 128, 3 * D1)),
    ("posx0", (S0, DK0, 128, ENC)), ("posx1", (S1, DK1, 128, ENC)),
]
PACK_F32 = [
    ("brz0", (128, 2 * DK0)), ("brz1", (128, 2 * DK1)),
    ("bihn0", (128, DK0)), ("bihn1", (128, DK1)),
    ("bhhn0", (128, DK0)), ("bhhn1", (128, DK1)),
    ("bemb0", (128, DK0)), ("bemb1", (128, DK1)),
    ("bpred0", (128, 1)), ("bpred1", (128, 1)),
]


def _offsets(spec):
    out, off = {}, 0
    for name, shape in spec:
        n = int(np.prod(shape))
        out[name] = (off, shape)
        off += n
    return out, off


OFF_BF, N_BF = _offsets(PACK_BF)
OFF_F8, N_F8 = _offsets(PACK_F8)
OFF_F32, N_F32 = _offsets(PACK_F32)
NY0, NY1 = S0 * SG0 * R, S1 * SG1 * R


def _build_nc(l0_steps=T0, l1_steps=T1):
    nc = bacc.Bacc("TRN2", target_bir_lowering=False, debug=False,
                   num_devices=NCORE)

    # ---------------- DRAM tensors (packed) ----------------
    bb_d = nc.dram_tensor("bb", [N_BF], BF, kind="ExternalInput")
    b8_d = nc.dram_tensor("b8", [N_F8], F8, kind="ExternalInput")
    bf_d = nc.dram_tensor("bf", [N_F32], F32, kind="ExternalInput")
    yy_d = nc.dram_tensor("yy", [NY0 + NY1], F32, kind="ExternalOutput")

    def _view(blob, off, shape):
        n = int(np.prod(shape))
        ap = blob[off:off + n]
        if len(shape) == 2:
            return ap.rearrange("(a b) -> a b", a=shape[0])
        if len(shape) == 3:
            return ap.rearrange("(a b c) -> a b c", a=shape[0], b=shape[1])
        return ap.rearrange("(a b c d) -> a b c d",
                            a=shape[0], b=shape[1], c=shape[2])

    def vbf(name):
        off, shape = OFF_BF[name]
        return _view(bb_d, off, shape)

    def vf8(name):
        off, shape = OFF_F8[name]
        return _view(b8_d, off, shape)

    def vf32(name):
        off, shape = OFF_F32[name]
        return _view(bf_d, off, shape)

    xseg0_d = vbf("xseg0")
    xseg1_d = vbf("xseg1")
    wih_d = [vf8("wihT0"), vf8("wihT1")]
    whh_d = [vbf("whhT0"), vbf("whhT1")]
    wemb_d = [vbf("wembT0"), vbf("wembT1")]
    wpred_d = [vbf("wpredT0"), vbf("wpredT1")]
    brz_d = [vf32("brz0"), vf32("brz1")]
    bihn_d = [vf32("bihn0"), vf32("bihn1")]
    bhhn_d = [vf32("bhhn0"), vf32("bhhn1")]
    bemb_d = [vf32("bemb0"), vf32("bemb1")]
    bpred_d = [vf32("bpred0"), vf32("bpred1")]
    posx_d = [vf8("posx0"), vf8("posx1")]
    ident_d = vbf("ident")
    y_d = [yy_d[0:NY0].rearrange("(a b c) -> a b c", a=S0, b=SG0),
           yy_d[NY0:NY0 + NY1].rearrange("(a b c) -> a b c", a=S1, b=SG1)]

    with tile.TileContext(nc) as tc:
        with tc.tile_pool(name="const", bufs=1) as cp, \
             tc.tile_pool(name="x0p", bufs=2) as x0p, \
             tc.tile_pool(name="xep", bufs=6) as xep, \
             tc.tile_pool(name="h0p", bufs=8) as h0p, \
             tc.tile_pool(name="h1p", bufs=4) as h1p, \
             tc.tile_pool(name="posp", bufs=5) as posp, \
             tc.tile_pool(name="hyp", bufs=6) as hyp, \
             tc.tile_pool(name="rp", bufs=4) as rp, \
             tc.tile_pool(name="zp", bufs=4) as zp, \
             tc.tile_pool(name="np_", bufs=4) as np_p, \
             tc.tile_pool(name="scp", bufs=4) as scp, \
             tc.tile_pool(name="tp", bufs=4) as tp, \
             tc.tile_pool(name="up", bufs=4) as up, \
             tc.tile_pool(name="vp", bufs=4) as vp, \
             tc.tile_pool(name="yp", bufs=2) as yp, \
             tc.tile_pool(name="psg", bufs=6, space="PSUM") as psg, \
             tc.tile_pool(name="psy", bufs=2, space="PSUM") as psy:

            # ---------------- load constants ----------------
            def load_w(dram, k_tiles, cols, nm, dt=BF):
                t = cp.tile([128, k_tiles * cols], dt, tag=f"c_{nm}",
                            name=f"c_{nm}")
                for k in range(k_tiles):
                    nc.sync.dma_start(t[:, k * cols:(k + 1) * cols], dram[k])
                return t

            wih_sb = [load_w(wih_d[0], DK0, 3 * D0, "wih0", F8),
                      load_w(wih_d[1], DK1, 3 * D1, "wih1", F8)]
            whh_sb = [load_w(whh_d[0], DK0, 3 * D0, "whh0"),
                      load_w(whh_d[1], DK1, 3 * D1, "whh1")]
            wpred_sb = [load_w(wpred_d[0], DK0, SG0, "wpred0"),
                        load_w(wpred_d[1], DK1, SG1, "wpred1")]
            wemb_sb = []
            for li, (sg, d) in enumerate(((SG0, D0), (SG1, D1))):
                t = cp.tile([sg, d], BF, tag=f"c_wemb{li}", name=f"c_wemb{li}")
                nc.sync.dma_start(t[:], wemb_d[li][:])
                wemb_sb.append(t)
            def load_b(dram, cols, nm):
                t = cp.tile([128, cols], F32, tag=f"c_{nm}", name=f"c_{nm}")
                nc.sync.dma_start(t[:], dram[:])
                return t
            brz_sb = [load_b(brz_d[0], 2 * DK0, "brz0"), load_b(brz_d[1], 2 * DK1, "brz1")]
            bihn_sb = [load_b(bihn_d[0], DK0, "bihn0"), load_b(bihn_d[1], DK1, "bihn1")]
            bhhn_sb = [load_b(bhhn_d[0], DK0, "bhhn0"), load_b(bhhn_d[1], DK1, "bhhn1")]
            bemb_sb = [load_b(bemb_d[0], DK0, "bemb0"), load_b(bemb_d[1], DK1, "bemb1")]
            bpred_sb = [load_b(bpred_d[0], 1, "bpred0"), load_b(bpred_d[1], 1, "bpred1")]
            xs1 = cp.tile([SG1, 4 * R], BF, tag="c_xs1", name="c_xs1")
            nc.sync.dma_start(xs1[:], xseg1_d[:])
            ident = cp.tile([128, 128], BF, tag="c_ident", name="c_ident")
            nc.sync.dma_start(ident[:], ident_d[:])
            # L1 x-side gate cache: gi1[j][m] = (Wih1 @ relu(emb(xs1_j)))[m]
            # for the 4 unique input segments, m over the 6 output 128-blocks.
            n_var = min(4, l1_steps)
            gi1 = [[cp.tile([128, R], BF, tag=f"c_gi1_{j}_{m}",
                            name=f"c_gi1_{j}_{m}")
                    for m in range(3 * DK1)] for j in range(n_var)]

            LP = [dict(D=D0, DK=DK0, SG=SG0, wih=wih_sb[0], whh=whh_sb[0],
                       wemb=wemb_sb[0], wpred=wpred_sb[0], brz=brz_sb[0],
                       bihn=bihn_sb[0], bhhn=bhhn_sb[0], bemb=bemb_sb[0],
                       bpred=bpred_sb[0]),
                  dict(D=D1, DK=DK1, SG=SG1, wih=wih_sb[1], whh=whh_sb[1],
                       wemb=wemb_sb[1], wpred=wpred_sb[1], brz=brz_sb[1],
                       bihn=bihn_sb[1], bhhn=bhhn_sb[1], bemb=bemb_sb[1],
                       bpred=bpred_sb[1])]

            def wcol(P, wt, k, m):
                """AP of [128,128] weight block: k-tile k, m-tile m of 3d."""
                c0 = k * 3 * P["D"] + m * 128
                return wt[:, c0:c0 + 128]

            def wpair(P, q, m):
                """DoubleRow lhsT AP [128, 2, 128]: k-tiles (2q, 2q+1)."""
                w3 = P["wih"][:].rearrange("p (k c) -> p k c", k=P["DK"])
                return w3[:, 2 * q:2 * q + 2, m * 128:(m + 1) * 128]

            def make_xe_embed(li, xsrc_fn):
                """Returns make_xe(c): per-chunk embed into fp8 pair tiles.

                Returns DK//2 DoubleRow rhs APs [128, 2, CH]."""
                P = LP[li]
                def make_xe(c):
                    aps = []
                    for q in range(P["DK"] // 2):
                        xe = xep.tile([128, 2 * CHPAD], F8, tag="xe",
                                      name=f"xe{li}_{q}")
                        for j in range(2):
                            k = 2 * q + j
                            ps = psg.tile([128, CH], F32, tag="ps", name="ps_e")
                            nc.tensor.matmul(ps[:], P["wemb"][:, k * 128:(k + 1) * 128],
                                             xsrc_fn(c), start=True, stop=True)
                            nc.scalar.activation(
                                xe[:, j * CHPAD:j * CHPAD + CH], ps[:],
                                AF.Relu, bias=P["bemb"][:, k:k + 1])
                        aps.append(xe[:].rearrange("p (j n) -> p j n", j=2)
                                   [:, :, 0:CH])
                    return aps
                return make_xe

            def emit_gru(li, make_xe, h_in, hout, first, gi_cache=None):
                """One fused GRU application over all chunks/blocks.

                make_xe(c) -> list of DK x-side rhs APs [128, CH] (unused when
                gi_cache is given).
                h_in: list of DK [128, R] tiles (prev h), or None if first.
                hout(i) -> [128, R] output AP for block i.
                gi_cache: list of 3*DK [128, R] SBUF bf16 tiles with the
                precomputed x-side pre-activations (biases NOT included).
                """
                P = LP[li]
                DK = P["DK"]
                for c in range(NCH):
                    cc = slice(c * CH, (c + 1) * CH)
                    xe = None if gi_cache is not None else make_xe(c)

                    def xacc(ps, m, close):
                        """x-side of gate-block m into ps (opens the group)."""
                        if gi_cache is not None:
                            nc.tensor.matmul(ps[:], ident[:], gi_cache[m][:, cc],
                                             start=True, stop=close)
                        else:
                            nq = DK // 2
                            for q in range(nq):
                                nc.tensor.matmul(ps[:], wpair(P, q, m), xe[q],
                                                 start=(q == 0),
                                                 stop=(q == nq - 1 and close),
                                                 perf_mode=DR)
                    for i in range(DK):
                        # --- r gate (m = i) ---
                        ps_r = psg.tile([128, CH], F32, tag="ps", name="ps_r")
                        xacc(ps_r, i, first)
                        if not first:
                            for k in range(DK):
                                nc.tensor.matmul(ps_r[:], wcol(P, P["whh"], k, i),
                                                 h_in[k][:, cc], start=False,
                                                 stop=(k == DK - 1))
                        r = rp.tile([128, CH], BF, tag="r", name="r_t")
                        nc.scalar.activation(r[:], ps_r[:], AF.Sigmoid,
                                             bias=P["brz"][:, i:i + 1],
                                             scale=SINV)
                        # --- z gate (m = DK + i) ---
                        ps_z = psg.tile([128, CH], F32, tag="ps", name="ps_z")
                        xacc(ps_z, DK + i, first)
                        if not first:
                            for k in range(DK):
                                nc.tensor.matmul(ps_z[:], wcol(P, P["whh"], k, DK + i),
                                                 h_in[k][:, cc], start=False,
                                                 stop=(k == DK - 1))
                        z = zp.tile([128, CH], BF, tag="z", name="z_t")
                        nc.scalar.activation(z[:], ps_z[:], AF.Sigmoid,
                                             bias=P["brz"][:, DK + i:DK + i + 1],
                                             scale=SINV)
                        # --- n gate: t = (gh_n + bhh_n) * r ---
                        t_ = tp.tile([128, CH], BF, tag="t", name="t_t")
                        if first:
                            nc.vector.tensor_scalar(t_[:], r[:],
                                                    P["bhhn"][:, i:i + 1], None,
                                                    op0=ALU.mult)
                        else:
                            ps_hn = psg.tile([128, CH], F32, tag="ps", name="ps_hn")
                            for k in range(DK):
                                nc.tensor.matmul(ps_hn[:], wcol(P, P["whh"], k, 2 * DK + i),
                                                 h_in[k][:, cc], start=(k == 0),
                                                 stop=(k == DK - 1))
                            nc.vector.scalar_tensor_tensor(
                                t_[:], ps_hn[:], P["bhhn"][:, i:i + 1], r[:],
                                op0=ALU.add, op1=ALU.mult)
                        # --- s = t + gi_n ; n = tanh(s + bih_n) ---
                        s_ = scp.tile([128, CH], BF, tag="s", name="s_t")
                        if gi_cache is not None:
                            nc.vector.tensor_add(s_[:], t_[:],
                                                 gi_cache[2 * DK + i][:, cc])
                        else:
                            ps_in = psg.tile([128, CH], F32, tag="ps", name="ps_in")
                            xacc(ps_in, 2 * DK + i, True)
                            nc.vector.tensor_add(s_[:], t_[:], ps_in[:])
                        n = np_p.tile([128, CH], BF, tag="n", name="n_t")
                        nc.scalar.activation(n[:], s_[:], AF.Tanh,
                                             bias=P["bihn"][:, i:i + 1],
                                             scale=SINV)
                        # --- h' = n + z*(h-n)  (h=0 when first) ---
                        if first:
                            v = vp.tile([128, CH], BF, tag="v", name="v_t")
                            nc.vector.tensor_mul(v[:], n[:], z[:])
                            nc.vector.tensor_sub(hout(i)[:, cc], n[:], v[:])
                        else:
                            u = up.tile([128, CH], BF, tag="u", name="u_t")
                            nc.gpsimd.tensor_sub(u[:], h_in[i][:, cc], n[:])
                            v = vp.tile([128, CH], BF, tag="v", name="v_t")
                            nc.gpsimd.tensor_mul(v[:], u[:], z[:])
                            nc.vector.tensor_add(hout(i)[:, cc], n[:], v[:])

            def emit_enc_step(li, t, make_xe, h_in, gi_cache=None):
                P = LP[li]
                h_pool = h0p if li == 0 else h1p
                h_out = [h_pool.tile([128, R], BF, tag=f"h{li}", name=f"h{li}_{t}_{k}")
                         for k in range(P["DK"])]
                emit_gru(li, make_xe, h_in, lambda i: h_out[i][:],
                         first=(t == 0), gi_cache=gi_cache)
                return h_out

            def emit_l1_cache_fill(j):
                """Compute gi1[j][m] = Wih1 @ relu(emb(xs1 seg j)) into SBUF."""
                P = LP[1]
                make_xe = make_xe_embed(
                    1, lambda c, j=j: xs1[:, j * R + c * CH:j * R + (c + 1) * CH])
                for c in range(NCH):
                    cc = slice(c * CH, (c + 1) * CH)
                    xe = make_xe(c)
                    for m in range(3 * DK1):
                        ps = psg.tile([128, CH], F32, tag="ps", name="ps_gi")
                        nc.tensor.matmul(ps[:], wpair(P, 0, m), xe[0],
                                         start=True, stop=True, perf_mode=DR)
                        nc.vector.tensor_copy(gi1[j][m][:, cc], ps[:])

            def emit_decoder(li, s_, h_fin):
                P = LP[li]
                DK, SG = P["DK"], P["SG"]
                hy = [hyp.tile([128, R], BF, tag="hy", name=f"hy{li}_{s_}_{i}")
                      for i in range(DK)]
                # full-R pos tiles; the 4x batch replication happens here via
                # 4 reads of the same unreplicated [128, ENC] DRAM slice.
                pts = []
                for q in range(DK // 2):
                    pt = posp.tile([128, 2 * RPAD], F8, tag="pos",
                                   name=f"pos{li}_{s_}_{q}")
                    for jj in range(2):
                        k = 2 * q + jj
                        for rep in range(BPC):
                            nc.sync.dma_start(
                                pt[:, jj * RPAD + rep * ENC:
                                   jj * RPAD + (rep + 1) * ENC],
                                posx_d[li][s_, k])
                    pts.append(pt[:].rearrange("p (j r) -> p j r", j=2))
                def make_xe(c):
                    return [pts[q][:, :, c * CH:(c + 1) * CH]
                            for q in range(DK // 2)]
                emit_gru(li, make_xe, h_fin, lambda i: hy[i][:], first=False)
                for c in range(NCH):
                    cc = slice(c * CH, (c + 1) * CH)
                    ps = psy.tile([SG, CH], F32, tag="psy", name="ps_y")
                    for k in range(DK):
                        nc.tensor.matmul(ps[:], P["wpred"][:, k * SG:(k + 1) * SG],
                                         hy[k][:, cc], start=(k == 0),
                                         stop=(k == DK - 1))
                    y = yp.tile([SG, CH], F32, tag="y", name="y_t")
                    nc.scalar.activation(y[:], ps[:], AF.Identity,
                                         bias=P["bpred"][0:SG, 0:1])
                    nc.sync.dma_start(y_d[li][s_, :, cc], y[:])

            # ---------------- encoder ----------------
            h0 = None
            h1 = None
            t1 = 0
            for t in range(l0_steps):
                xs_t = x0p.tile([SG0, R], BF, tag="xs0", name=f"xs0_{t}")
                nc.sync.dma_start(xs_t[:], xseg0_d[t])
                h0 = emit_enc_step(
                    0, t, make_xe_embed(0, lambda c, xs_t=xs_t: xs_t[:, c * CH:(c + 1) * CH]),
                    h0)
                for _ in range(4):
                    if t1 < l1_steps:
                        j = t1 % 4
                        if t1 < n_var:
                            emit_l1_cache_fill(j)
                        h1 = emit_enc_step(1, t1, None, h1, gi_cache=gi1[j])
                        t1 += 1
            while t1 < l1_steps:
                j = t1 % 4
                if t1 < n_var:
                    emit_l1_cache_fill(j)
                h1 = emit_enc_step(1, t1, None, h1, gi_cache=gi1[j])
                t1 += 1

            # ---------------- decoders ----------------
            emit_decoder(0, 0, h0)
            emit_decoder(1, 0, h1)
            emit_decoder(0, 1, h0)
            emit_decoder(1, 1, h1)
            emit_decoder(1, 2, h1)
            emit_decoder(1, 3, h1)

    nc.compile()
    return nc


def get_nc(l0_steps=T0, l1_steps=T1):
    key = (l0_steps, l1_steps)
    if key not in _CACHE:
        _CACHE[key] = _build_nc(l0_steps, l1_steps)
    return _CACHE[key]


# ==================== host side ====================

BF_NP = mybir.dt.np(mybir.dt.bfloat16)
F8_NP = mybir.dt.np(mybir.dt.float8e4)


def _prep_shared(inp):
    f = np.float32
    m = {}
    for li, d in ((0, D0), (1, D1)):
        dk = (DK0, DK1)[li]
        sg = (SG0, SG1)[li]
        m[f"wembT{li}"] = np.ascontiguousarray(
            inp[f"W_emb{li}"].T * XS).astype(BF_NP)
        m[f"wihT{li}"] = np.ascontiguousarray(
            inp[f"Wih{li}"].T.reshape(dk, 128, 3 * d) * WS).astype(F8_NP)
        m[f"whhT{li}"] = np.ascontiguousarray(
            inp[f"Whh{li}"].T.reshape(dk, 128, 3 * d) * S_SC).astype(BF_NP)
        m[f"wpredT{li}"] = np.ascontiguousarray(
            inp[f"Wpred{li}"].T.reshape(dk, 128, sg)).astype(BF_NP)
        bih, bhh = inp[f"bih{li}"].astype(f), inp[f"bhh{li}"].astype(f)
        m[f"brz{li}"] = np.ascontiguousarray(
            (bih + bhh)[:2 * d].reshape(2 * dk, 128).T)
        m[f"bihn{li}"] = np.ascontiguousarray(bih[2 * d:].reshape(dk, 128).T)
        m[f"bhhn{li}"] = np.ascontiguousarray(
            bhh[2 * d:].reshape(dk, 128).T * S_SC)
        m[f"bemb{li}"] = np.ascontiguousarray(
            inp[f"b_emb{li}"].astype(f).reshape(dk, 128).T * XS)
        bp = np.zeros((128, 1), f)
        bp[:sg, 0] = inp[f"bpred{li}"].astype(f)
        m[f"bpred{li}"] = bp
        half = d // 2
        pos, chan = inp[f"pos{li}"].astype(f), inp[f"chan{li}"].astype(f)
        S = pos.shape[0]
        base = np.concatenate(
            [np.broadcast_to(pos[:, None, :], (S, ENC, half)),
             np.broadcast_to(chan[None, :, :], (S, ENC, half))], axis=-1)
        posx = base.transpose(0, 2, 1) * XS                   # [S, d, ENC]
        m[f"posx{li}"] = np.ascontiguousarray(
            posx.reshape(S, dk, 128, ENC)).astype(F8_NP)
    m["ident"] = np.eye(128, dtype=BF_NP)
    return m


def _prep_core(x, c):
    f = np.float32
    xb = x[BPC * c:BPC * (c + 1)].astype(f)
    last = xb[:, -1:, :]
    xc = (xb - last).transpose(0, 2, 1).reshape(R, SEQ)
    xseg0 = np.ascontiguousarray(
        xc.reshape(R, T0, SG0).transpose(1, 2, 0)).astype(BF_NP)
    xseg1 = np.ascontiguousarray(
        xc[:, :4 * SG1].reshape(R, 4, SG1).transpose(2, 1, 0).reshape(SG1, 4 * R)
    ).astype(BF_NP)
    return xseg0, xseg1


def make_in_maps(inp):
    """Build per-core packed input maps ({'bb': .., 'bf': ..})."""
    x = np.asarray(inp["x"], np.float32)
    shared = _prep_shared({k: np.asarray(v) for k, v in inp.items()})
    bf = np.empty(N_F32, np.float32)
    for name, shape in PACK_F32:
        off, _ = OFF_F32[name]
        bf[off:off + int(np.prod(shape))] = shared[name].ravel()
    bb_tail = np.empty(N_BF, BF_NP)
    for name, shape in PACK_BF:
        if name in ("xseg0", "xseg1"):
            continue
        off, _ = OFF_BF[name]
        bb_tail[off:off + int(np.prod(shape))] = shared[name].ravel()
    b8 = np.empty(N_F8, F8_NP)
    for name, shape in PACK_F8:
        off, _ = OFF_F8[name]
        b8[off:off + int(np.prod(shape))] = shared[name].ravel()
    in_maps = []
    for c in range(NCORE):
        xseg0, xseg1 = _prep_core(x, c)
        bb = bb_tail.copy()
        o0, _ = OFF_BF["xseg0"]
        bb[o0:o0 + xseg0.size] = xseg0.ravel()
        o1, _ = OFF_BF["xseg1"]
        bb[o1:o1 + xseg1.size] = xseg1.ravel()
        in_maps.append({"bb": bb, "b8": b8, "bf": bf})
    return in_maps


def split_y(yy_core):
    """Split one core's packed output into (y0, y1)."""
    y0 = yy_core[:NY0].reshape(S0, SG0, R)
    y1 = yy_core[NY0:NY0 + NY1].reshape(S1, SG1, R)
    return y0, y1


def assemble_output(yy_per_core, x):
    """yy_per_core: list of 8 flat yy arrays -> full [B, PRED, ENC] output."""
    ys = [split_y(np.asarray(yy).ravel()) for yy in yy_per_core]
    full0 = np.concatenate([y0 for y0, _ in ys], axis=2)
    full1 = np.concatenate([y1 for _, y1 in ys], axis=2)
    # out[b, s_*seg+j, e] = y[s_, j, n=(b,e)]
    yl0 = full0.reshape(S0, SG0, B, ENC).transpose(2, 0, 1, 3).reshape(B, PRED, ENC)
    yl1 = full1.reshape(S1, SG1, B, ENC).transpose(2, 0, 1, 3).reshape(B, PRED, ENC)
    return ((yl0 + yl1) / 2.0 + x[:, -1:, :]).astype(np.float32)


def kernel(**inputs):
    x = np.asarray(inputs["x"], np.float32)
    in_maps = make_in_maps(inputs)
    nc = get_nc()
    res = run_bass_kernel_spmd(nc, in_maps, list(range(NCORE))).results
    return assemble_output([res[c]["yy"] for c in range(NCORE)], x)


# revision 5
# speedup vs baseline: 2.5525x; 1.1729x over previous
"""Trainium2 Bass kernel for nn_Hierarch_RNN (hierarchical 2-layer GRU), v2.

Changes vs v1 baseline:
  - bf16 for all matmul operands + elementwise state (PSUM stays f32):
    2x DVE throughput on SBUF tensor-tensor ops, halved SBUF/DMA traffic,
    same PE rate as f32r.
  - Layer-1 input is periodic with period 4 (segments 0..3 repeat 15x),
    so the x-side gate pre-activations gi = Wih1 @ relu(emb) are computed
    once per unique segment and cached in SBUF; the 56 repeated steps
    initialize the r/z PSUM with an identity matmul from the cache and the
    n-gate folds the cached term into the existing DVE add. Saves 30 of 78
    matmuls per L1 step and all embed work.
  - Embed relu moved from ScalarE to a fused DVE tensor_scalar
    (add-bias, max-0) to balance engine load (ScalarE was near-critical).
  - L1 tanh batched over the full row dim (one act per 128-block instead
    of three) to amortize the ~350-cycle ACT instruction overhead.
"""
import numpy as np

import concourse.mybir as mybir
import concourse.tile as tile
from concourse import bacc
from concourse.bass_utils import run_bass_kernel_spmd

F32 = mybir.dt.float32
BF = mybir.dt.bfloat16
F8 = mybir.dt.float8e4
AF = mybir.ActivationFunctionType
ALU = mybir.AluOpType
DR = mybir.MatmulPerfMode.DoubleRow

# fp8 scale folding: Wih ships as fp8 * WS, xe/pos as fp8 * XS (XS folded
# into Wemb/bemb host-side), Whh ships as bf16 * WS*XS, so every gate PSUM
# is uniformly scaled by S = WS*XS and the activations descale via scale=.
WS, XS = 256.0, 32.0
S_SC = WS * XS
SINV = 1.0 / S_SC
RPAD = 1296               # %16-aligned j-stride for DoubleRow rhs (>= R)
CHPAD = 432               # %16-aligned j-stride for xe pair tiles (>= CH)

B, SEQ, PRED, ENC = 32, 720, 96, 321
NCORE, BPC = 8, 4
R = BPC * ENC                 # 1284 rows per core
CH, NCH = 428, 3              # row chunks (428 f32 <= 512/bank)
# layer params: d, seg_len, n 128-blocks of d (DK), decoder steps S
D0, SG0, DK0, S0, T0 = 512, 48, 4, 2, 15
D1, SG1, DK1, S1, T1 = 256, 24, 2, 4, 60

_CACHE = {}

# Packed-input layouts: all bf16 tensors in one flat blob, all f32 in
# another — per-argument dispatch overhead through the exec path is large,
# so the kernel takes 2 inputs + 1 output instead of ~31/2.
# posx ships unreplicated ([.., 321] instead of [.., R=4*321]); the 4x
# batch replication happens via 4 on-device DMA reads.
PACK_BF = [
    ("xseg0", (T0, SG0, R)),
    ("xseg1", (SG1, 4 * R)),
    ("wembT0", (SG0, D0)), ("wembT1", (SG1, D1)),
    ("wpredT0", (DK0, 128, SG0)), ("wpredT1", (DK1, 128, SG1)),
    ("ident", (128, 128)),
]
PACK_F8 = [
    ("wihT0", (DK0, 128, 3 * D0)), ("wihT1", (DK1, 128, 3 * D1)),
    ("whhT0", (DK0, 128, 3 * D0)), ("whhT1", (DK1, 128, 3 * D1)),
    ("posx0", (S0, DK0, 128, ENC)), ("posx1", (S1, DK1, 128, ENC)),
]
PACK_F32 = [
    ("brz0", (128, 2 * DK0)), ("brz1", (128, 2 * DK1)),
    ("bihn0", (128, DK0)), ("bihn1", (128, DK1)),
    ("bhhn0", (128, DK0)), ("bhhn1", (128, DK1)),
    ("bemb0", (128, DK0)), ("bemb1", (128, DK1)),
    ("bpred0", (128, 1)), ("bpred1", (128, 1)),
]


def _offsets(spec):
    out, off = {}, 0
    for name, shape in spec:
        n = int(np.prod(shape))
        out[name] = (off, shape)
        off += n
    return out, off


OFF_BF, N_BF = _offsets(PACK_BF)
OFF_F8, N_F8 = _offsets(PACK_F8)
OFF_F32, N_F32 = _offsets(PACK_F32)
NY0, NY1 = S0 * SG0 * R, S1 * SG1 * R


def _build_nc(l0_steps=T0, l1_steps=T1):
    nc = bacc.Bacc("TRN2", target_bir_lowering=False, debug=False,
                   num_devices=NCORE)

    # ---------------- DRAM tensors (packed) ----------------
    bb_d = nc.dram_tensor("bb", [N_BF], BF, kind="ExternalInput")
    b8_d = nc.dram_tensor("b8", [N_F8], F8, kind="ExternalInput")
    bf_d = nc.dram_tensor("bf", [N_F32], F32, kind="ExternalInput")
    yy_d = nc.dram_tensor("yy", [NY0 + NY1], F32, kind="ExternalOutput")

    def _view(blob, off, shape):
        n = int(np.prod(shape))
        ap = blob[off:off + n]
        if len(shape) == 2:
            return ap.rearrange("(a b) -> a b", a=shape[0])
        if len(shape) == 3:
            return ap.rearrange("(a b c) -> a b c", a=shape[0], b=shape[1])
        return ap.rearrange("(a b c d) -> a b c d",
                            a=shape[0], b=shape[1], c=shape[2])

    def vbf(name):
        off, shape = OFF_BF[name]
        return _view(bb_d, off, shape)

    def vf8(name):
        off, shape = OFF_F8[name]
        return _view(b8_d, off, shape)

    def vf32(name):
        off, shape = OFF_F32[name]
        return _view(bf_d, off, shape)

    xseg0_d = vbf("xseg0")
    xseg1_d = vbf("xseg1")
    wih_d = [vf8("wihT0"), vf8("wihT1")]
    whh_d = [vf8("whhT0"), vf8("whhT1")]
    wemb_d = [vbf("wembT0"), vbf("wembT1")]
    wpred_d = [vbf("wpredT0"), vbf("wpredT1")]
    brz_d = [vf32("brz0"), vf32("brz1")]
    bihn_d = [vf32("bihn0"), vf32("bihn1")]
    bhhn_d = [vf32("bhhn0"), vf32("bhhn1")]
    bemb_d = [vf32("bemb0"), vf32("bemb1")]
    bpred_d = [vf32("bpred0"), vf32("bpred1")]
    posx_d = [vf8("posx0"), vf8("posx1")]
    ident_d = vbf("ident")
    y_d = [yy_d[0:NY0].rearrange("(a b c) -> a b c", a=S0, b=SG0),
           yy_d[NY0:NY0 + NY1].rearrange("(a b c) -> a b c", a=S1, b=SG1)]

    with tile.TileContext(nc) as tc:
        with tc.tile_pool(name="const", bufs=1) as cp, \
             tc.tile_pool(name="x0p", bufs=2) as x0p, \
             tc.tile_pool(name="xep", bufs=6) as xep, \
             tc.tile_pool(name="h0p", bufs=4) as h0p, \
             tc.tile_pool(name="h1p", bufs=2) as h1p, \
             tc.tile_pool(name="posp", bufs=5) as posp, \
             tc.tile_pool(name="hyp", bufs=6) as hyp, \
             tc.tile_pool(name="rp", bufs=4) as rp, \
             tc.tile_pool(name="zp", bufs=4) as zp, \
             tc.tile_pool(name="np_", bufs=4) as np_p, \
             tc.tile_pool(name="scp", bufs=4) as scp, \
             tc.tile_pool(name="tp", bufs=4) as tp, \
             tc.tile_pool(name="up", bufs=4) as up, \
             tc.tile_pool(name="vp", bufs=4) as vp, \
             tc.tile_pool(name="yp", bufs=2) as yp, \
             tc.tile_pool(name="psg", bufs=6, space="PSUM") as psg, \
             tc.tile_pool(name="psy", bufs=2, space="PSUM") as psy:

            # ---------------- load constants ----------------
            def load_w(dram, k_tiles, cols, nm, dt=BF):
                t = cp.tile([128, k_tiles * cols], dt, tag=f"c_{nm}",
                            name=f"c_{nm}")
                for k in range(k_tiles):
                    nc.sync.dma_start(t[:, k * cols:(k + 1) * cols], dram[k])
                return t

            wih_sb = [load_w(wih_d[0], DK0, 3 * D0, "wih0", F8),
                      load_w(wih_d[1], DK1, 3 * D1, "wih1", F8)]
            whh_sb = [load_w(whh_d[0], DK0, 3 * D0, "whh0", F8),
                      load_w(whh_d[1], DK1, 3 * D1, "whh1", F8)]
            wpred_sb = [load_w(wpred_d[0], DK0, SG0, "wpred0"),
                        load_w(wpred_d[1], DK1, SG1, "wpred1")]
            wemb_sb = []
            for li, (sg, d) in enumerate(((SG0, D0), (SG1, D1))):
                t = cp.tile([sg, d], BF, tag=f"c_wemb{li}", name=f"c_wemb{li}")
                nc.sync.dma_start(t[:], wemb_d[li][:])
                wemb_sb.append(t)
            def load_b(dram, cols, nm):
                t = cp.tile([128, cols], F32, tag=f"c_{nm}", name=f"c_{nm}")
                nc.sync.dma_start(t[:], dram[:])
                return t
            brz_sb = [load_b(brz_d[0], 2 * DK0, "brz0"), load_b(brz_d[1], 2 * DK1, "brz1")]
            bihn_sb = [load_b(bihn_d[0], DK0, "bihn0"), load_b(bihn_d[1], DK1, "bihn1")]
            bhhn_sb = [load_b(bhhn_d[0], DK0, "bhhn0"), load_b(bhhn_d[1], DK1, "bhhn1")]
            bemb_sb = [load_b(bemb_d[0], DK0, "bemb0"), load_b(bemb_d[1], DK1, "bemb1")]
            bpred_sb = [load_b(bpred_d[0], 1, "bpred0"), load_b(bpred_d[1], 1, "bpred1")]
            xs1 = cp.tile([SG1, 4 * R], BF, tag="c_xs1", name="c_xs1")
            nc.sync.dma_start(xs1[:], xseg1_d[:])
            ident = cp.tile([128, 128], BF, tag="c_ident", name="c_ident")
            nc.sync.dma_start(ident[:], ident_d[:])
            # L1 x-side gate cache: gi1[j][m] = (Wih1 @ relu(emb(xs1_j)))[m]
            # for the 4 unique input segments, m over the 6 output 128-blocks.
            n_var = min(4, l1_steps)
            gi1 = [[cp.tile([128, R], BF, tag=f"c_gi1_{j}_{m}",
                            name=f"c_gi1_{j}_{m}")
                    for m in range(3 * DK1)] for j in range(n_var)]

            LP = [dict(D=D0, DK=DK0, SG=SG0, wih=wih_sb[0], whh=whh_sb[0],
                       wemb=wemb_sb[0], wpred=wpred_sb[0], brz=brz_sb[0],
                       bihn=bihn_sb[0], bhhn=bhhn_sb[0], bemb=bemb_sb[0],
                       bpred=bpred_sb[0]),
                  dict(D=D1, DK=DK1, SG=SG1, wih=wih_sb[1], whh=whh_sb[1],
                       wemb=wemb_sb[1], wpred=wpred_sb[1], brz=brz_sb[1],
                       bihn=bihn_sb[1], bhhn=bhhn_sb[1], bemb=bemb_sb[1],
                       bpred=bpred_sb[1])]

            def wcol(P, wt, k, m):
                """AP of [128,128] weight block: k-tile k, m-tile m of 3d."""
                c0 = k * 3 * P["D"] + m * 128
                return wt[:, c0:c0 + 128]

            def wpair(P, q, m, wt=None):
                """DoubleRow lhsT AP [128, 2, 128]: k-tiles (2q, 2q+1)."""
                w3 = (P["wih"] if wt is None else wt)[:].rearrange(
                    "p (k c) -> p k c", k=P["DK"])
                return w3[:, 2 * q:2 * q + 2, m * 128:(m + 1) * 128]

            def make_xe_embed(li, xsrc_fn):
                """Returns make_xe(c): per-chunk embed into fp8 pair tiles.

                Returns DK//2 DoubleRow rhs APs [128, 2, CH]."""
                P = LP[li]
                def make_xe(c):
                    aps = []
                    for q in range(P["DK"] // 2):
                        xe = xep.tile([128, 2 * CHPAD], F8, tag="xe",
                                      name=f"xe{li}_{q}")
                        for j in range(2):
                            k = 2 * q + j
                            ps = psg.tile([128, CH], F32, tag="ps", name="ps_e")
                            nc.tensor.matmul(ps[:], P["wemb"][:, k * 128:(k + 1) * 128],
                                             xsrc_fn(c), start=True, stop=True)
                            nc.scalar.activation(
                                xe[:, j * CHPAD:j * CHPAD + CH], ps[:],
                                AF.Relu, bias=P["bemb"][:, k:k + 1])
                        aps.append(xe[:].rearrange("p (j n) -> p j n", j=2)
                                   [:, :, 0:CH])
                    return aps
                return make_xe

            def emit_gru(li, make_xe, h_in, hout, first, gi_cache=None,
                         fp8_h_out=False):
                """One fused GRU application over all chunks/blocks.

                make_xe(c) -> list of DK//2 DoubleRow rhs APs (unused when
                gi_cache is given).
                h_in: list of DK//2 fp8 pair tiles [128, 2*RPAD] (scaled x32),
                or None if first.
                hout(i) -> [128, R] output AP for block i; fp8 x32 slices of
                pair tiles when fp8_h_out, else plain bf16.
                gi_cache: list of 3*DK [128, R] SBUF bf16 tiles with the
                precomputed (scaled) x-side pre-activations.
                """
                P = LP[li]
                DK = P["DK"]

                def h_blk(i):
                    q, j = i // 2, i % 2
                    return h_in[q][:, j * RPAD:j * RPAD + R]

                def h_rhs(q, cc):
                    h3 = h_in[q][:].rearrange("p (j r) -> p j r", j=2)
                    return h3[:, :, cc]
                for c in range(NCH):
                    cc = slice(c * CH, (c + 1) * CH)
                    xe = None if gi_cache is not None else make_xe(c)

                    def xacc(ps, m, close):
                        """x-side of gate-block m into ps (opens the group)."""
                        if gi_cache is not None:
                            nc.tensor.matmul(ps[:], ident[:], gi_cache[m][:, cc],
                                             start=True, stop=close)
                        else:
                            nq = DK // 2
                            for q in range(nq):
                                nc.tensor.matmul(ps[:], wpair(P, q, m), xe[q],
                                                 start=(q == 0),
                                                 stop=(q == nq - 1 and close),
                                                 perf_mode=DR)
                    for i in range(DK):
                        # --- r gate (m = i) ---
                        ps_r = psg.tile([128, CH], F32, tag="ps", name="ps_r")
                        xacc(ps_r, i, first)
                        if not first:
                            nq = DK // 2
                            for q in range(nq):
                                nc.tensor.matmul(ps_r[:], wpair(P, q, i, P["whh"]),
                                                 h_rhs(q, cc), start=False,
                                                 stop=(q == nq - 1), perf_mode=DR)
                        r = rp.tile([128, CH], BF, tag="r", name="r_t")
                        nc.scalar.activation(r[:], ps_r[:], AF.Sigmoid,
                                             bias=P["brz"][:, i:i + 1],
                                             scale=SINV)
                        # --- z gate (m = DK + i) ---
                        ps_z = psg.tile([128, CH], F32, tag="ps", name="ps_z")
                        xacc(ps_z, DK + i, first)
                        if not first:
                            nq = DK // 2
                            for q in range(nq):
                                nc.tensor.matmul(ps_z[:], wpair(P, q, DK + i, P["whh"]),
                                                 h_rhs(q, cc), start=False,
                                                 stop=(q == nq - 1), perf_mode=DR)
                        z = zp.tile([128, CH], BF, tag="z", name="z_t")
                        nc.scalar.activation(z[:], ps_z[:], AF.Sigmoid,
                                             bias=P["brz"][:, DK + i:DK + i + 1],
                                             scale=SINV)
                        # --- n gate: t = (gh_n + bhh_n) * r ---
                        t_ = tp.tile([128, CH], BF, tag="t", name="t_t")
                        if first:
                            nc.vector.tensor_scalar(t_[:], r[:],
                                                    P["bhhn"][:, i:i + 1], None,
                                                    op0=ALU.mult)
                        else:
                            ps_hn = psg.tile([128, CH], F32, tag="ps", name="ps_hn")
                            nq = DK // 2
                            for q in range(nq):
                                nc.tensor.matmul(ps_hn[:], wpair(P, q, 2 * DK + i, P["whh"]),
                                                 h_rhs(q, cc), start=(q == 0),
                                                 stop=(q == nq - 1), perf_mode=DR)
                            nc.vector.scalar_tensor_tensor(
                                t_[:], ps_hn[:], P["bhhn"][:, i:i + 1], r[:],
                                op0=ALU.add, op1=ALU.mult)
                        # --- s = t + gi_n ; n = tanh(s + bih_n) ---
                        s_ = scp.tile([128, CH], BF, tag="s", name="s_t")
                        if gi_cache is not None:
                            nc.gpsimd.tensor_add(s_[:], t_[:],
                                                 gi_cache[2 * DK + i][:, cc])
                        else:
                            ps_in = psg.tile([128, CH], F32, tag="ps", name="ps_in")
                            xacc(ps_in, 2 * DK + i, True)
                            nc.vector.tensor_add(s_[:], t_[:], ps_in[:])
                        n = np_p.tile([128, CH], BF, tag="n", name="n_t")
                        nc.scalar.activation(n[:], s_[:], AF.Tanh,
                                             bias=P["bihn"][:, i:i + 1],
                                             scale=SINV)
                        # --- h' = n + z*(h-n)  (h=0 when first) ---
                        # encoder state is fp8 scaled x32: u32 = h32 - 32n,
                        # v32 = u32*z, h'32 = 32n + v32 (stt folds the x32).
                        if first:
                            v = vp.tile([128, CH], BF, tag="v", name="v_t")
                            nc.gpsimd.tensor_mul(v[:], n[:], z[:])
                            u = up.tile([128, CH], BF, tag="u", name="u_t")
                            nc.vector.tensor_sub(u[:], n[:], v[:])
                            if fp8_h_out:
                                nc.vector.tensor_scalar(hout(i)[:, cc], u[:],
                                                        XS, None, op0=ALU.mult)
                            else:
                                nc.vector.tensor_copy(hout(i)[:, cc], u[:])
                        elif fp8_h_out:
                            u = up.tile([128, CH], BF, tag="u", name="u_t")
                            nc.vector.scalar_tensor_tensor(
                                u[:], n[:], -XS, h_blk(i)[:, cc],
                                op0=ALU.mult, op1=ALU.add)
                            v = vp.tile([128, CH], BF, tag="v", name="v_t")
                            nc.gpsimd.tensor_mul(v[:], u[:], z[:])
                            nc.vector.scalar_tensor_tensor(
                                hout(i)[:, cc], n[:], XS, v[:],
                                op0=ALU.mult, op1=ALU.add)
                        else:
                            u = up.tile([128, CH], BF, tag="u", name="u_t")
                            nc.vector.scalar_tensor_tensor(
                                u[:], h_blk(i)[:, cc], 1.0 / XS, n[:],
                                op0=ALU.mult, op1=ALU.subtract)
                            v = vp.tile([128, CH], BF, tag="v", name="v_t")
                            nc.gpsimd.tensor_mul(v[:], u[:], z[:])
                            nc.vector.tensor_add(hout(i)[:, cc], n[:], v[:])

            def emit_enc_step(li, t, make_xe, h_in, gi_cache=None):
                P = LP[li]
                h_pool = h0p if li == 0 else h1p
                h_out = [h_pool.tile([128, 2 * RPAD], F8, tag=f"h{li}",
                                     name=f"h{li}_{t}_{q}")
                         for q in range(P["DK"] // 2)]
                def hout(i):
                    q, j = i // 2, i % 2
                    return h_out[q][:, j * RPAD:j * RPAD + R]
                emit_gru(li, make_xe, h_in, hout,
                         first=(t == 0), gi_cache=gi_cache, fp8_h_out=True)
                return h_out

            def emit_l1_cache_fill(j):
                """Compute gi1[j][m] = Wih1 @ relu(emb(xs1 seg j)) into SBUF."""
                P = LP[1]
                make_xe = make_xe_embed(
                    1, lambda c, j=j: xs1[:, j * R + c * CH:j * R + (c + 1) * CH])
                for c in range(NCH):
                    cc = slice(c * CH, (c + 1) * CH)
                    xe = make_xe(c)
                    for m in range(3 * DK1):
                        ps = psg.tile([128, CH], F32, tag="ps", name="ps_gi")
                        nc.tensor.matmul(ps[:], wpair(P, 0, m), xe[0],
                                         start=True, stop=True, perf_mode=DR)
                        nc.vector.tensor_copy(gi1[j][m][:, cc], ps[:])

            def emit_decoder(li, s_, h_fin):
                P = LP[li]
                DK, SG = P["DK"], P["SG"]
                hy = [hyp.tile([128, R], BF, tag="hy", name=f"hy{li}_{s_}_{i}")
                      for i in range(DK)]
                # full-R pos tiles; the 4x batch replication happens here via
                # 4 reads of the same unreplicated [128, ENC] DRAM slice.
                pts = []
                for q in range(DK // 2):
                    pt = posp.tile([128, 2 * RPAD], F8, tag="pos",
                                   name=f"pos{li}_{s_}_{q}")
                    for jj in range(2):
                        k = 2 * q + jj
                        for rep in range(BPC):
                            nc.sync.dma_start(
                                pt[:, jj * RPAD + rep * ENC:
                                   jj * RPAD + (rep + 1) * ENC],
                                posx_d[li][s_, k])
                    pts.append(pt[:].rearrange("p (j r) -> p j r", j=2))
                def make_xe(c):
                    return [pts[q][:, :, c * CH:(c + 1) * CH]
                            for q in range(DK // 2)]
                emit_gru(li, make_xe, h_fin, lambda i: hy[i][:], first=False)
                for c in range(NCH):
                    cc = slice(c * CH, (c + 1) * CH)
                    ps = psy.tile([SG, CH], F32, tag="psy", name="ps_y")
                    for k in range(DK):
                        nc.tensor.matmul(ps[:], P["wpred"][:, k * SG:(k + 1) * SG],
                                         hy[k][:, cc], start=(k == 0),
                                         stop=(k == DK - 1))
                    y = yp.tile([SG, CH], F32, tag="y", name="y_t")
                    nc.scalar.activation(y[:], ps[:], AF.Identity,
                                         bias=P["bpred"][0:SG, 0:1])
                    nc.sync.dma_start(y_d[li][s_, :, cc], y[:])

            # ---------------- encoder ----------------
            h0 = None
            h1 = None
            t1 = 0
            for t in range(l0_steps):
                xs_t = x0p.tile([SG0, R], BF, tag="xs0", name=f"xs0_{t}")
                nc.sync.dma_start(xs_t[:], xseg0_d[t])
                h0 = emit_enc_step(
                    0, t, make_xe_embed(0, lambda c, xs_t=xs_t: xs_t[:, c * CH:(c + 1) * CH]),
                    h0)
                for _ in range(4):
                    if t1 < l1_steps:
                        j = t1 % 4
                        if t1 < n_var:
                            emit_l1_cache_fill(j)
                        h1 = emit_enc_step(1, t1, None, h1, gi_cache=gi1[j])
                        t1 += 1
            while t1 < l1_steps:
                j = t1 % 4
                if t1 < n_var:
                    emit_l1_cache_fill(j)
                h1 = emit_enc_step(1, t1, None, h1, gi_cache=gi1[j])
                t1 += 1

            # ---------------- decoders ----------------
            emit_decoder(0, 0, h0)
            emit_decoder(1, 0, h1)
            emit_decoder(0, 1, h0)
            emit_decoder(1, 1, h1)
            emit_decoder(1, 2, h1)
            emit_decoder(1, 3, h1)

    nc.compile()
    return nc


def get_nc(l0_steps=T0, l1_steps=T1):
    key = (l0_steps, l1_steps)
    if key not in _CACHE:
        _CACHE[key] = _build_nc(l0_steps, l1_steps)
    return _CACHE[key]


# ==================== host side ====================

BF_NP = mybir.dt.np(mybir.dt.bfloat16)
F8_NP = mybir.dt.np(mybir.dt.float8e4)


def _prep_shared(inp):
    f = np.float32
    m = {}
    for li, d in ((0, D0), (1, D1)):
        dk = (DK0, DK1)[li]
        sg = (SG0, SG1)[li]
        m[f"wembT{li}"] = np.ascontiguousarray(
            inp[f"W_emb{li}"].T * XS).astype(BF_NP)
        m[f"wihT{li}"] = np.ascontiguousarray(
            inp[f"Wih{li}"].T.reshape(dk, 128, 3 * d) * WS).astype(F8_NP)
        m[f"whhT{li}"] = np.ascontiguousarray(
            inp[f"Whh{li}"].T.reshape(dk, 128, 3 * d) * WS).astype(F8_NP)
        m[f"wpredT{li}"] = np.ascontiguousarray(
            inp[f"Wpred{li}"].T.reshape(dk, 128, sg)).astype(BF_NP)
        bih, bhh = inp[f"bih{li}"].astype(f), inp[f"bhh{li}"].astype(f)
        m[f"brz{li}"] = np.ascontiguousarray(
            (bih + bhh)[:2 * d].reshape(2 * dk, 128).T)
        m[f"bihn{li}"] = np.ascontiguousarray(bih[2 * d:].reshape(dk, 128).T)
        m[f"bhhn{li}"] = np.ascontiguousarray(
            bhh[2 * d:].reshape(dk, 128).T * S_SC)
        m[f"bemb{li}"] = np.ascontiguousarray(
            inp[f"b_emb{li}"].astype(f).reshape(dk, 128).T * XS)
        bp = np.zeros((128, 1), f)
        bp[:sg, 0] = inp[f"bpred{li}"].astype(f)
        m[f"bpred{li}"] = bp
        half = d // 2
        pos, chan = inp[f"pos{li}"].astype(f), inp[f"chan{li}"].astype(f)
        S = pos.shape[0]
        base = np.concatenate(
            [np.broadcast_to(pos[:, None, :], (S, ENC, half)),
             np.broadcast_to(chan[None, :, :], (S, ENC, half))], axis=-1)
        posx = base.transpose(0, 2, 1) * XS                   # [S, d, ENC]
        m[f"posx{li}"] = np.ascontiguousarray(
            posx.reshape(S, dk, 128, ENC)).astype(F8_NP)
    m["ident"] = np.eye(128, dtype=BF_NP)
    return m


def _prep_core(x, c):
    f = np.float32
    xb = x[BPC * c:BPC * (c + 1)].astype(f)
    last = xb[:, -1:, :]
    xc = (xb - last).transpose(0, 2, 1).reshape(R, SEQ)
    xseg0 = np.ascontiguousarray(
        xc.reshape(R, T0, SG0).transpose(1, 2, 0)).astype(BF_NP)
    xseg1 = np.ascontiguousarray(
        xc[:, :4 * SG1].reshape(R, 4, SG1).transpose(2, 1, 0).reshape(SG1, 4 * R)
    ).astype(BF_NP)
    return xseg0, xseg1


def make_in_maps(inp):
    """Build per-core packed input maps ({'bb': .., 'bf': ..})."""
    x = np.asarray(inp["x"], np.float32)
    shared = _prep_shared({k: np.asarray(v) for k, v in inp.items()})
    bf = np.empty(N_F32, np.float32)
    for name, shape in PACK_F32:
        off, _ = OFF_F32[name]
        bf[off:off + int(np.prod(shape))] = shared[name].ravel()
    bb_tail = np.empty(N_BF, BF_NP)
    for name, shape in PACK_BF:
        if name in ("xseg0", "xseg1"):
            continue
        off, _ = OFF_BF[name]
        bb_tail[off:off + int(np.prod(shape))] = shared[name].ravel()
    b8 = np.empty(N_F8, F8_NP)
    for name, shape in PACK_F8:
        off, _ = OFF_F8[name]
        b8[off:off + int(np.prod(shape))] = shared[name].ravel()
    in_maps = []
    for c in range(NCORE):
        xseg0, xseg1 = _prep_core(x, c)
        bb = bb_tail.copy()
        o0, _ = OFF_BF["xseg0"]
        bb[o0:o0 + xseg0.size] = xseg0.ravel()
        o1, _ = OFF_BF["xseg1"]
        bb[o1:o1 + xseg1.size] = xseg1.ravel()
        in_maps.append({"bb": bb, "b8": b8, "bf": bf})
    return in_maps


def split_y(yy_core):
    """Split one core's packed output into (y0, y1)."""
    y0 = yy_core[:NY0].reshape(S0, SG0, R)
    y1 = yy_core[NY0:NY0 + NY1].reshape(S1, SG1, R)
    return y0, y1


def assemble_output(yy_per_core, x):
    """yy_per_core: list of 8 flat yy arrays -> full [B, PRED, ENC] output."""
    ys = [split_y(np.asarray(yy).ravel()) for yy in yy_per_core]
    full0 = np.concatenate([y0 for y0, _ in ys], axis=2)
    full1 = np.concatenate([y1 for _, y1 in ys], axis=2)
    # out[b, s_*seg+j, e] = y[s_, j, n=(b,e)]
    yl0 = full0.reshape(S0, SG0, B, ENC).transpose(2, 0, 1, 3).reshape(B, PRED, ENC)
    yl1 = full1.reshape(S1, SG1, B, ENC).transpose(2, 0, 1, 3).reshape(B, PRED, ENC)
    return ((yl0 + yl1) / 2.0 + x[:, -1:, :]).astype(np.float32)


def kernel(**inputs):
    x = np.asarray(inputs["x"], np.float32)
    in_maps = make_in_maps(inputs)
    nc = get_nc()
    res = run_bass_kernel_spmd(nc, in_maps, list(range(NCORE))).results
    return assemble_output([res[c]["yy"] for c in range(NCORE)], x)


# revision 6
# speedup vs baseline: 2.7875x; 1.0921x over previous
"""Trainium2 Bass kernel for nn_Hierarch_RNN (hierarchical 2-layer GRU), v2.

Changes vs v1 baseline:
  - bf16 for all matmul operands + elementwise state (PSUM stays f32):
    2x DVE throughput on SBUF tensor-tensor ops, halved SBUF/DMA traffic,
    same PE rate as f32r.
  - Layer-1 input is periodic with period 4 (segments 0..3 repeat 15x),
    so the x-side gate pre-activations gi = Wih1 @ relu(emb) are computed
    once per unique segment and cached in SBUF; the 56 repeated steps
    initialize the r/z PSUM with an identity matmul from the cache and the
    n-gate folds the cached term into the existing DVE add. Saves 30 of 78
    matmuls per L1 step and all embed work.
  - Embed relu moved from ScalarE to a fused DVE tensor_scalar
    (add-bias, max-0) to balance engine load (ScalarE was near-critical).
  - L1 tanh batched over the full row dim (one act per 128-block instead
    of three) to amortize the ~350-cycle ACT instruction overhead.
"""
import numpy as np

import concourse.mybir as mybir
import concourse.tile as tile
from concourse import bacc
from concourse.bass_utils import run_bass_kernel_spmd

F32 = mybir.dt.float32
BF = mybir.dt.bfloat16
F8 = mybir.dt.float8e4
AF = mybir.ActivationFunctionType
ALU = mybir.AluOpType
DR = mybir.MatmulPerfMode.DoubleRow

# fp8 scale folding: Wih ships as fp8 * WS, xe/pos as fp8 * XS (XS folded
# into Wemb/bemb host-side), Whh ships as bf16 * WS*XS, so every gate PSUM
# is uniformly scaled by S = WS*XS and the activations descale via scale=.
WS, XS = 256.0, 32.0
S_SC = WS * XS
SINV = 1.0 / S_SC
RPAD = 1296               # %16-aligned j-stride for DoubleRow rhs (>= R)
CHPAD = 432               # %16-aligned j-stride for xe pair tiles (>= CH)

B, SEQ, PRED, ENC = 32, 720, 96, 321
NCORE, BPC = 8, 4
R = BPC * ENC                 # 1284 rows per core
CH, NCH = 428, 3              # row chunks (428 f32 <= 512/bank)
# layer params: d, seg_len, n 128-blocks of d (DK), decoder steps S
D0, SG0, DK0, S0, T0 = 512, 48, 4, 2, 15
D1, SG1, DK1, S1, T1 = 256, 24, 2, 4, 60

_CACHE = {}

# Packed-input layouts: all bf16 tensors in one flat blob, all f32 in
# another — per-argument dispatch overhead through the exec path is large,
# so the kernel takes 2 inputs + 1 output instead of ~31/2.
# posx ships unreplicated ([.., 321] instead of [.., R=4*321]); the 4x
# batch replication happens via 4 on-device DMA reads.
PACK_BF = [
    ("xseg0", (T0, SG0, R)),
    ("xseg1", (SG1, 4 * R)),
    ("wembT0", (SG0, D0)), ("wembT1", (SG1, D1)),
    ("wpredT0", (DK0, 128, SG0)), ("wpredT1", (DK1, 128, SG1)),
    ("ident", (128, 128)),
]
PACK_F8 = [
    ("wihT0", (DK0, 128, 3 * D0)), ("wihT1", (DK1, 128, 3 * D1)),
    ("whhT0", (DK0, 128, 3 * D0)), ("whhT1", (DK1, 128, 3 * D1)),
    ("posx0", (S0, DK0, 128, ENC)), ("posx1", (S1, DK1, 128, ENC)),
]
PACK_F32 = [
    ("brz0", (128, 2 * DK0)), ("brz1", (128, 2 * DK1)),
    ("brzs0", (128, 2 * DK0)), ("brzs1", (128, 2 * DK1)),
    ("bihn0", (128, DK0)), ("bihn1", (128, DK1)),
    ("bhhn0", (128, DK0)), ("bhhn1", (128, DK1)),
    ("bemb0", (128, DK0)), ("bemb1", (128, DK1)),
    ("bpred0", (128, 1)), ("bpred1", (128, 1)),
]


def _offsets(spec):
    out, off = {}, 0
    for name, shape in spec:
        n = int(np.prod(shape))
        out[name] = (off, shape)
        off += n
    return out, off


OFF_BF, N_BF = _offsets(PACK_BF)
OFF_F8, N_F8 = _offsets(PACK_F8)
OFF_F32, N_F32 = _offsets(PACK_F32)
NY0, NY1 = S0 * SG0 * R, S1 * SG1 * R


def _build_nc(l0_steps=T0, l1_steps=T1):
    nc = bacc.Bacc("TRN2", target_bir_lowering=False, debug=False,
                   num_devices=NCORE)

    # ---------------- DRAM tensors (packed) ----------------
    bb_d = nc.dram_tensor("bb", [N_BF], BF, kind="ExternalInput")
    b8_d = nc.dram_tensor("b8", [N_F8], F8, kind="ExternalInput")
    bf_d = nc.dram_tensor("bf", [N_F32], F32, kind="ExternalInput")
    yy_d = nc.dram_tensor("yy", [NY0 + NY1], F32, kind="ExternalOutput")

    def _view(blob, off, shape):
        n = int(np.prod(shape))
        ap = blob[off:off + n]
        if len(shape) == 2:
            return ap.rearrange("(a b) -> a b", a=shape[0])
        if len(shape) == 3:
            return ap.rearrange("(a b c) -> a b c", a=shape[0], b=shape[1])
        return ap.rearrange("(a b c d) -> a b c d",
                            a=shape[0], b=shape[1], c=shape[2])

    def vbf(name):
        off, shape = OFF_BF[name]
        return _view(bb_d, off, shape)

    def vf8(name):
        off, shape = OFF_F8[name]
        return _view(b8_d, off, shape)

    def vf32(name):
        off, shape = OFF_F32[name]
        return _view(bf_d, off, shape)

    xseg0_d = vbf("xseg0")
    xseg1_d = vbf("xseg1")
    wih_d = [vf8("wihT0"), vf8("wihT1")]
    whh_d = [vf8("whhT0"), vf8("whhT1")]
    wemb_d = [vbf("wembT0"), vbf("wembT1")]
    wpred_d = [vbf("wpredT0"), vbf("wpredT1")]
    brz_d = [vf32("brz0"), vf32("brz1")]
    brzs_d = [vf32("brzs0"), vf32("brzs1")]
    bihn_d = [vf32("bihn0"), vf32("bihn1")]
    bhhn_d = [vf32("bhhn0"), vf32("bhhn1")]
    bemb_d = [vf32("bemb0"), vf32("bemb1")]
    bpred_d = [vf32("bpred0"), vf32("bpred1")]
    posx_d = [vf8("posx0"), vf8("posx1")]
    ident_d = vbf("ident")
    y_d = [yy_d[0:NY0].rearrange("(a b c) -> a b c", a=S0, b=SG0),
           yy_d[NY0:NY0 + NY1].rearrange("(a b c) -> a b c", a=S1, b=SG1)]

    with tile.TileContext(nc) as tc:
        with tc.tile_pool(name="const", bufs=1) as cp, \
             tc.tile_pool(name="x0p", bufs=2) as x0p, \
             tc.tile_pool(name="xep", bufs=6) as xep, \
             tc.tile_pool(name="h0p", bufs=4) as h0p, \
             tc.tile_pool(name="h1p", bufs=2) as h1p, \
             tc.tile_pool(name="posp", bufs=5) as posp, \
             tc.tile_pool(name="hyp", bufs=6) as hyp, \
             tc.tile_pool(name="rp", bufs=4) as rp, \
             tc.tile_pool(name="zp", bufs=4) as zp, \
             tc.tile_pool(name="np_", bufs=4) as np_p, \
             tc.tile_pool(name="scp", bufs=4) as scp, \
             tc.tile_pool(name="tp", bufs=4) as tp, \
             tc.tile_pool(name="up", bufs=4) as up, \
             tc.tile_pool(name="vp", bufs=4) as vp, \
             tc.tile_pool(name="yp", bufs=2) as yp, \
             tc.tile_pool(name="rzsb", bufs=4) as rzsb, \
             tc.tile_pool(name="psg", bufs=4, space="PSUM") as psg, \
             tc.tile_pool(name="rz2", bufs=2, space="PSUM") as rz2:

            # ---------------- load constants ----------------
            def load_w(dram, k_tiles, cols, nm, dt=BF):
                t = cp.tile([128, k_tiles * cols], dt, tag=f"c_{nm}",
                            name=f"c_{nm}")
                for k in range(k_tiles):
                    nc.sync.dma_start(t[:, k * cols:(k + 1) * cols], dram[k])
                return t

            wih_sb = [load_w(wih_d[0], DK0, 3 * D0, "wih0", F8),
                      load_w(wih_d[1], DK1, 3 * D1, "wih1", F8)]
            whh_sb = [load_w(whh_d[0], DK0, 3 * D0, "whh0", F8),
                      load_w(whh_d[1], DK1, 3 * D1, "whh1", F8)]
            wpred_sb = [load_w(wpred_d[0], DK0, SG0, "wpred0"),
                        load_w(wpred_d[1], DK1, SG1, "wpred1")]
            wemb_sb = []
            for li, (sg, d) in enumerate(((SG0, D0), (SG1, D1))):
                t = cp.tile([sg, d], BF, tag=f"c_wemb{li}", name=f"c_wemb{li}")
                nc.sync.dma_start(t[:], wemb_d[li][:])
                wemb_sb.append(t)
            def load_b(dram, cols, nm):
                t = cp.tile([128, cols], F32, tag=f"c_{nm}", name=f"c_{nm}")
                nc.sync.dma_start(t[:], dram[:])
                return t
            brz_sb = [load_b(brz_d[0], 2 * DK0, "brz0"), load_b(brz_d[1], 2 * DK1, "brz1")]
            brzs_sb = [load_b(brzs_d[0], 2 * DK0, "brzs0"), load_b(brzs_d[1], 2 * DK1, "brzs1")]
            bihn_sb = [load_b(bihn_d[0], DK0, "bihn0"), load_b(bihn_d[1], DK1, "bihn1")]
            bhhn_sb = [load_b(bhhn_d[0], DK0, "bhhn0"), load_b(bhhn_d[1], DK1, "bhhn1")]
            bemb_sb = [load_b(bemb_d[0], DK0, "bemb0"), load_b(bemb_d[1], DK1, "bemb1")]
            bpred_sb = [load_b(bpred_d[0], 1, "bpred0"), load_b(bpred_d[1], 1, "bpred1")]
            xs1 = cp.tile([SG1, 4 * R], BF, tag="c_xs1", name="c_xs1")
            nc.sync.dma_start(xs1[:], xseg1_d[:])
            ident = cp.tile([128, 128], BF, tag="c_ident", name="c_ident")
            nc.sync.dma_start(ident[:], ident_d[:])
            # L1 x-side gate cache: gi1[j][m] = (Wih1 @ relu(emb(xs1_j)))[m]
            # for the 4 unique input segments, m over the 6 output 128-blocks.
            n_var = min(4, l1_steps)
            gi1 = [[cp.tile([128, R], BF, tag=f"c_gi1_{j}_{m}",
                            name=f"c_gi1_{j}_{m}")
                    for m in range(3 * DK1)] for j in range(n_var)]

            LP = [dict(D=D0, DK=DK0, SG=SG0, wih=wih_sb[0], whh=whh_sb[0],
                       wemb=wemb_sb[0], wpred=wpred_sb[0], brz=brz_sb[0],
                       brzs=brzs_sb[0],
                       bihn=bihn_sb[0], bhhn=bhhn_sb[0], bemb=bemb_sb[0],
                       bpred=bpred_sb[0]),
                  dict(D=D1, DK=DK1, SG=SG1, wih=wih_sb[1], whh=whh_sb[1],
                       wemb=wemb_sb[1], wpred=wpred_sb[1], brz=brz_sb[1],
                       brzs=brzs_sb[1],
                       bihn=bihn_sb[1], bhhn=bhhn_sb[1], bemb=bemb_sb[1],
                       bpred=bpred_sb[1])]

            def wcol(P, wt, k, m):
                """AP of [128,128] weight block: k-tile k, m-tile m of 3d."""
                c0 = k * 3 * P["D"] + m * 128
                return wt[:, c0:c0 + 128]

            def wpair(P, q, m, wt=None):
                """DoubleRow lhsT AP [128, 2, 128]: k-tiles (2q, 2q+1)."""
                w3 = (P["wih"] if wt is None else wt)[:].rearrange(
                    "p (k c) -> p k c", k=P["DK"])
                return w3[:, 2 * q:2 * q + 2, m * 128:(m + 1) * 128]

            def make_xe_embed(li, xsrc_fn):
                """Returns make_xe(c): per-chunk embed into fp8 pair tiles.

                Returns DK//2 DoubleRow rhs APs [128, 2, CH]."""
                P = LP[li]
                def make_xe(c):
                    aps = []
                    for q in range(P["DK"] // 2):
                        xe = xep.tile([128, 2 * CHPAD], F8, tag="xe",
                                      name=f"xe{li}_{q}")
                        for j in range(2):
                            k = 2 * q + j
                            ps = psg.tile([128, CH], F32, tag="ps", name="ps_e")
                            nc.tensor.matmul(ps[:], P["wemb"][:, k * 128:(k + 1) * 128],
                                             xsrc_fn(c), start=True, stop=True)
                            nc.scalar.activation(
                                xe[:, j * CHPAD:j * CHPAD + CH], ps[:],
                                AF.Relu, bias=P["bemb"][:, k:k + 1])
                        aps.append(xe[:].rearrange("p (j n) -> p j n", j=2)
                                   [:, :, 0:CH])
                    return aps
                return make_xe

            def emit_gru(li, make_xe, h_in, hout, first, gi_cache=None,
                         fp8_h_out=False):
                """One fused GRU application over all chunks/blocks.

                make_xe(c) -> list of DK//2 DoubleRow rhs APs (unused when
                gi_cache is given).
                h_in: list of DK//2 fp8 pair tiles [128, 2*RPAD] (scaled x32),
                or None if first.
                hout(i) -> [128, R] output AP for block i; fp8 x32 slices of
                pair tiles when fp8_h_out, else plain bf16.
                gi_cache: list of 3*DK [128, R] SBUF bf16 tiles with the
                precomputed (scaled) x-side pre-activations.
                """
                P = LP[li]
                DK = P["DK"]

                def h_blk(i):
                    q, j = i // 2, i % 2
                    return h_in[q][:, j * RPAD:j * RPAD + R]

                def h_rhs(q, cc):
                    h3 = h_in[q][:].rearrange("p (j r) -> p j r", j=2)
                    return h3[:, :, cc]
                for c in range(NCH):
                    cc = slice(c * CH, (c + 1) * CH)
                    xe = None if gi_cache is not None else make_xe(c)

                    def xacc(ps, m, close):
                        """x-side of gate-block m into ps (opens the group)."""
                        if gi_cache is not None:
                            nc.tensor.matmul(ps, ident[:], gi_cache[m][:, cc],
                                             start=True, stop=close)
                        else:
                            nq = DK // 2
                            for q in range(nq):
                                nc.tensor.matmul(ps, wpair(P, q, m), xe[q],
                                                 start=(q == 0),
                                                 stop=(q == nq - 1 and close),
                                                 perf_mode=DR)
                    for i in range(DK):
                        if gi_cache is not None:
                            # --- merged r/z: 2-bank psum pair, one act ---
                            # (brz is already baked into the cache)
                            prz = rz2.tile([128, 1024], F32, tag="rz",
                                           name="ps_rz")
                            for gbase, m in ((0, i), (512, DK + i)):
                                half = prz[:, gbase:gbase + CH]
                                nc.tensor.matmul(half, ident[:],
                                                 gi_cache[m][:, cc],
                                                 start=True, stop=first)
                                if not first:
                                    nq = DK // 2
                                    for q in range(nq):
                                        nc.tensor.matmul(
                                            half, wpair(P, q, m, P["whh"]),
                                            h_rhs(q, cc), start=False,
                                            stop=(q == nq - 1), perf_mode=DR)
                            rzt = rzsb.tile([128, 2 * CH], BF, tag="rz",
                                            name="rz_t")
                            in3 = prz[:].rearrange("p (g n) -> p g n",
                                                   g=2)[:, :, 0:CH]
                            out3 = rzt[:].rearrange("p (g n) -> p g n", g=2)
                            nc.scalar.activation(out3, in3, AF.Sigmoid,
                                                 bias=0.0, scale=SINV)
                            r = rzt[:, 0:CH]
                            z = rzt[:, CH:2 * CH]
                        else:
                            # --- r gate (m = i) ---
                            prz = rz2.tile([128, 1024], F32, tag="rz",
                                           name="ps_rz")
                            ps_r = prz[:, 0:CH]
                            xacc(ps_r, i, first)
                            if not first:
                                nq = DK // 2
                                for q in range(nq):
                                    nc.tensor.matmul(ps_r, wpair(P, q, i, P["whh"]),
                                                     h_rhs(q, cc), start=False,
                                                     stop=(q == nq - 1), perf_mode=DR)
                            r_t = rp.tile([128, CH], BF, tag="r", name="r_t")
                            nc.scalar.activation(r_t[:], ps_r, AF.Sigmoid,
                                                 bias=P["brz"][:, i:i + 1],
                                                 scale=SINV)
                            r = r_t[:]
                            # --- z gate (m = DK + i) ---
                            ps_z = prz[:, 512:512 + CH]
                            xacc(ps_z, DK + i, first)
                            if not first:
                                nq = DK // 2
                                for q in range(nq):
                                    nc.tensor.matmul(ps_z, wpair(P, q, DK + i, P["whh"]),
                                                     h_rhs(q, cc), start=False,
                                                     stop=(q == nq - 1), perf_mode=DR)
                            z_t = zp.tile([128, CH], BF, tag="z", name="z_t")
                            nc.scalar.activation(z_t[:], ps_z, AF.Sigmoid,
                                                 bias=P["brz"][:, DK + i:DK + i + 1],
                                                 scale=SINV)
                            z = z_t[:]
                        # --- n gate: t = (gh_n + bhh_n) * r ---
                        t_ = tp.tile([128, CH], BF, tag="t", name="t_t")
                        if first:
                            nc.vector.tensor_scalar(t_[:], r,
                                                    P["bhhn"][:, i:i + 1], None,
                                                    op0=ALU.mult)
                        else:
                            ps_hn = psg.tile([128, CH], F32, tag="ps", name="ps_hn")
                            nq = DK // 2
                            for q in range(nq):
                                nc.tensor.matmul(ps_hn[:], wpair(P, q, 2 * DK + i, P["whh"]),
                                                 h_rhs(q, cc), start=(q == 0),
                                                 stop=(q == nq - 1), perf_mode=DR)
                            nc.vector.scalar_tensor_tensor(
                                t_[:], ps_hn[:], P["bhhn"][:, i:i + 1], r,
                                op0=ALU.add, op1=ALU.mult)
                        # --- s = t + gi_n ; n = tanh(s + bih_n) ---
                        s_ = scp.tile([128, CH], BF, tag="s", name="s_t")
                        if gi_cache is not None:
                            nc.gpsimd.tensor_add(s_[:], t_[:],
                                                 gi_cache[2 * DK + i][:, cc])
                        else:
                            ps_in = psg.tile([128, CH], F32, tag="ps", name="ps_in")
                            xacc(ps_in[:], 2 * DK + i, True)
                            nc.vector.tensor_add(s_[:], t_[:], ps_in[:])
                        n = np_p.tile([128, CH], BF, tag="n", name="n_t")
                        nc.scalar.activation(n[:], s_[:], AF.Tanh,
                                             bias=P["bihn"][:, i:i + 1],
                                             scale=SINV)
                        # --- h' = n + z*(h-n)  (h=0 when first) ---
                        # encoder state is fp8 scaled x32: u32 = h32 - 32n,
                        # v32 = u32*z, h'32 = 32n + v32 (stt folds the x32).
                        if first:
                            v = vp.tile([128, CH], BF, tag="v", name="v_t")
                            nc.gpsimd.tensor_mul(v[:], n[:], z)
                            u = up.tile([128, CH], BF, tag="u", name="u_t")
                            nc.vector.tensor_sub(u[:], n[:], v[:])
                            if fp8_h_out:
                                nc.vector.tensor_scalar(hout(i)[:, cc], u[:],
                                                        XS, None, op0=ALU.mult)
                            else:
                                nc.vector.tensor_copy(hout(i)[:, cc], u[:])
                        elif fp8_h_out:
                            u = up.tile([128, CH], BF, tag="u", name="u_t")
                            nc.vector.scalar_tensor_tensor(
                                u[:], n[:], -XS, h_blk(i)[:, cc],
                                op0=ALU.mult, op1=ALU.add)
                            v = vp.tile([128, CH], BF, tag="v", name="v_t")
                            nc.gpsimd.tensor_mul(v[:], u[:], z)
                            nc.vector.scalar_tensor_tensor(
                                hout(i)[:, cc], n[:], XS, v[:],
                                op0=ALU.mult, op1=ALU.add)
                        else:
                            u = up.tile([128, CH], BF, tag="u", name="u_t")
                            nc.vector.scalar_tensor_tensor(
                                u[:], h_blk(i)[:, cc], 1.0 / XS, n[:],
                                op0=ALU.mult, op1=ALU.subtract)
                            v = vp.tile([128, CH], BF, tag="v", name="v_t")
                            nc.gpsimd.tensor_mul(v[:], u[:], z)
                            nc.vector.tensor_add(hout(i)[:, cc], n[:], v[:])

            def emit_enc_step(li, t, make_xe, h_in, gi_cache=None):
                P = LP[li]
                h_pool = h0p if li == 0 else h1p
                h_out = [h_pool.tile([128, 2 * RPAD], F8, tag=f"h{li}",
                                     name=f"h{li}_{t}_{q}")
                         for q in range(P["DK"] // 2)]
                def hout(i):
                    q, j = i // 2, i % 2
                    return h_out[q][:, j * RPAD:j * RPAD + R]
                emit_gru(li, make_xe, h_in, hout,
                         first=(t == 0), gi_cache=gi_cache, fp8_h_out=True)
                return h_out

            def emit_l1_cache_fill(j):
                """Compute gi1[j][m] = Wih1 @ relu(emb(xs1 seg j)) into SBUF."""
                P = LP[1]
                make_xe = make_xe_embed(
                    1, lambda c, j=j: xs1[:, j * R + c * CH:j * R + (c + 1) * CH])
                for c in range(NCH):
                    cc = slice(c * CH, (c + 1) * CH)
                    xe = make_xe(c)
                    for m in range(3 * DK1):
                        ps = psg.tile([128, CH], F32, tag="ps", name="ps_gi")
                        nc.tensor.matmul(ps[:], wpair(P, 0, m), xe[0],
                                         start=True, stop=True, perf_mode=DR)
                        if m < 2 * DK1:
                            nc.vector.tensor_scalar(
                                gi1[j][m][:, cc], ps[:],
                                P["brzs"][:, m:m + 1], None, op0=ALU.add)
                        else:
                            nc.vector.tensor_copy(gi1[j][m][:, cc], ps[:])

            def emit_decoder(li, s_, h_fin):
                P = LP[li]
                DK, SG = P["DK"], P["SG"]
                hy = [hyp.tile([128, R], BF, tag="hy", name=f"hy{li}_{s_}_{i}")
                      for i in range(DK)]
                # full-R pos tiles; the 4x batch replication happens here via
                # 4 reads of the same unreplicated [128, ENC] DRAM slice.
                pts = []
                for q in range(DK // 2):
                    pt = posp.tile([128, 2 * RPAD], F8, tag="pos",
                                   name=f"pos{li}_{s_}_{q}")
                    for jj in range(2):
                        k = 2 * q + jj
                        for rep in range(BPC):
                            nc.sync.dma_start(
                                pt[:, jj * RPAD + rep * ENC:
                                   jj * RPAD + (rep + 1) * ENC],
                                posx_d[li][s_, k])
                    pts.append(pt[:].rearrange("p (j r) -> p j r", j=2))
                def make_xe(c):
                    return [pts[q][:, :, c * CH:(c + 1) * CH]
                            for q in range(DK // 2)]
                emit_gru(li, make_xe, h_fin, lambda i: hy[i][:], first=False)
                for c in range(NCH):
                    cc = slice(c * CH, (c + 1) * CH)
                    ps_full = psg.tile([128, CH], F32, tag="ps", name="ps_y")
                    ps = ps_full[0:SG, :]
                    for k in range(DK):
                        nc.tensor.matmul(ps, P["wpred"][:, k * SG:(k + 1) * SG],
                                         hy[k][:, cc], start=(k == 0),
                                         stop=(k == DK - 1))
                    y = yp.tile([SG, CH], F32, tag="y", name="y_t")
                    nc.scalar.activation(y[:], ps, AF.Identity,
                                         bias=P["bpred"][0:SG, 0:1])
                    nc.sync.dma_start(y_d[li][s_, :, cc], y[:])

            # ---------------- encoder ----------------
            h0 = None
            h1 = None
            t1 = 0
            for t in range(l0_steps):
                xs_t = x0p.tile([SG0, R], BF, tag="xs0", name=f"xs0_{t}")
                nc.sync.dma_start(xs_t[:], xseg0_d[t])
                h0 = emit_enc_step(
                    0, t, make_xe_embed(0, lambda c, xs_t=xs_t: xs_t[:, c * CH:(c + 1) * CH]),
                    h0)
                for _ in range(4):
                    if t1 < l1_steps:
                        j = t1 % 4
                        if t1 < n_var:
                            emit_l1_cache_fill(j)
                        h1 = emit_enc_step(1, t1, None, h1, gi_cache=gi1[j])
                        t1 += 1
            while t1 < l1_steps:
                j = t1 % 4
                if t1 < n_var:
                    emit_l1_cache_fill(j)
                h1 = emit_enc_step(1, t1, None, h1, gi_cache=gi1[j])
                t1 += 1

            # ---------------- decoders ----------------
            emit_decoder(0, 0, h0)
            emit_decoder(1, 0, h1)
            emit_decoder(0, 1, h0)
            emit_decoder(1, 1, h1)
            emit_decoder(1, 2, h1)
            emit_decoder(1, 3, h1)

    nc.compile()
    return nc


def get_nc(l0_steps=T0, l1_steps=T1):
    key = (l0_steps, l1_steps)
    if key not in _CACHE:
        _CACHE[key] = _build_nc(l0_steps, l1_steps)
    return _CACHE[key]


# ==================== host side ====================

BF_NP = mybir.dt.np(mybir.dt.bfloat16)
F8_NP = mybir.dt.np(mybir.dt.float8e4)


def _prep_shared(inp):
    f = np.float32
    m = {}
    for li, d in ((0, D0), (1, D1)):
        dk = (DK0, DK1)[li]
        sg = (SG0, SG1)[li]
        m[f"wembT{li}"] = np.ascontiguousarray(
            inp[f"W_emb{li}"].T * XS).astype(BF_NP)
        m[f"wihT{li}"] = np.ascontiguousarray(
            inp[f"Wih{li}"].T.reshape(dk, 128, 3 * d) * WS).astype(F8_NP)
        m[f"whhT{li}"] = np.ascontiguousarray(
            inp[f"Whh{li}"].T.reshape(dk, 128, 3 * d) * WS).astype(F8_NP)
        m[f"wpredT{li}"] = np.ascontiguousarray(
            inp[f"Wpred{li}"].T.reshape(dk, 128, sg)).astype(BF_NP)
        bih, bhh = inp[f"bih{li}"].astype(f), inp[f"bhh{li}"].astype(f)
        m[f"brz{li}"] = np.ascontiguousarray(
            (bih + bhh)[:2 * d].reshape(2 * dk, 128).T)
        m[f"brzs{li}"] = m[f"brz{li}"] * S_SC
        m[f"bihn{li}"] = np.ascontiguousarray(bih[2 * d:].reshape(dk, 128).T)
        m[f"bhhn{li}"] = np.ascontiguousarray(
            bhh[2 * d:].reshape(dk, 128).T * S_SC)
        m[f"bemb{li}"] = np.ascontiguousarray(
            inp[f"b_emb{li}"].astype(f).reshape(dk, 128).T * XS)
        bp = np.zeros((128, 1), f)
        bp[:sg, 0] = inp[f"bpred{li}"].astype(f)
        m[f"bpred{li}"] = bp
        half = d // 2
        pos, chan = inp[f"pos{li}"].astype(f), inp[f"chan{li}"].astype(f)
        S = pos.shape[0]
        base = np.concatenate(
            [np.broadcast_to(pos[:, None, :], (S, ENC, half)),
             np.broadcast_to(chan[None, :, :], (S, ENC, half))], axis=-1)
        posx = base.transpose(0, 2, 1) * XS                   # [S, d, ENC]
        m[f"posx{li}"] = np.ascontiguousarray(
            posx.reshape(S, dk, 128, ENC)).astype(F8_NP)
    m["ident"] = np.eye(128, dtype=BF_NP)
    return m


def _prep_core(x, c):
    f = np.float32
    xb = x[BPC * c:BPC * (c + 1)].astype(f)
    last = xb[:, -1:, :]
    xc = (xb - last).transpose(0, 2, 1).reshape(R, SEQ)
    xseg0 = np.ascontiguousarray(
        xc.reshape(R, T0, SG0).transpose(1, 2, 0)).astype(BF_NP)
    xseg1 = np.ascontiguousarray(
        xc[:, :4 * SG1].reshape(R, 4, SG1).transpose(2, 1, 0).reshape(SG1, 4 * R)
    ).astype(BF_NP)
    return xseg0, xseg1


def make_in_maps(inp):
    """Build per-core packed input maps ({'bb': .., 'bf': ..})."""
    x = np.asarray(inp["x"], np.float32)
    shared = _prep_shared({k: np.asarray(v) for k, v in inp.items()})
    bf = np.empty(N_F32, np.float32)
    for name, shape in PACK_F32:
        off, _ = OFF_F32[name]
        bf[off:off + int(np.prod(shape))] = shared[name].ravel()
    bb_tail = np.empty(N_BF, BF_NP)
    for name, shape in PACK_BF:
        if name in ("xseg0", "xseg1"):
            continue
        off, _ = OFF_BF[name]
        bb_tail[off:off + int(np.prod(shape))] = shared[name].ravel()
    b8 = np.empty(N_F8, F8_NP)
    for name, shape in PACK_F8:
        off, _ = OFF_F8[name]
        b8[off:off + int(np.prod(shape))] = shared[name].ravel()
    in_maps = []
    for c in range(NCORE):
        xseg0, xseg1 = _prep_core(x, c)
        bb = bb_tail.copy()
        o0, _ = OFF_BF["xseg0"]
        bb[o0:o0 + xseg0.size] = xseg0.ravel()
        o1, _ = OFF_BF["xseg1"]
        bb[o1:o1 + xseg1.size] = xseg1.ravel()
        in_maps.append({"bb": bb, "b8": b8, "bf": bf})
    return in_maps


def split_y(yy_core):
    """Split one core's packed output into (y0, y1)."""
    y0 = yy_core[:NY0].reshape(S0, SG0, R)
    y1 = yy_core[NY0:NY0 + NY1].reshape(S1, SG1, R)
    return y0, y1


def assemble_output(yy_per_core, x):
    """yy_per_core: list of 8 flat yy arrays -> full [B, PRED, ENC] output."""
    ys = [split_y(np.asarray(yy).ravel()) for yy in yy_per_core]
    full0 = np.concatenate([y0 for y0, _ in ys], axis=2)
    full1 = np.concatenate([y1 for _, y1 in ys], axis=2)
    # out[b, s_*seg+j, e] = y[s_, j, n=(b,e)]
    yl0 = full0.reshape(S0, SG0, B, ENC).transpose(2, 0, 1, 3).reshape(B, PRED, ENC)
    yl1 = full1.reshape(S1, SG1, B, ENC).transpose(2, 0, 1, 3).reshape(B, PRED, ENC)
    return ((yl0 + yl1) / 2.0 + x[:, -1:, :]).astype(np.float32)


def kernel(**inputs):
    x = np.asarray(inputs["x"], np.float32)
    in_maps = make_in_maps(inputs)
    nc = get_nc()
    res = run_bass_kernel_spmd(nc, in_maps, list(range(NCORE))).results
    return assemble_output([res[c]["yy"] for c in range(NCORE)], x)
